# revision 44
# baseline (speedup 1.0000x reference)
"""GAT+GCN+pool GNN on 8 Trainium2 NeuronCores (Bass/Tile), fp8 edition.

Sharding: nodes/edges partitioned across 8 cores by destination-node range;
segment softmax and scatter-adds are core-local.  Per-edge row gathers use
dma_gather on fp8 rows (h stored as [2496 h | 32 a_src | 32 a_dst] fp8e4),
scatter-adds are DoubleRow fp8 one-hot matmuls (256 edges per pass).

GCN is computed as (A_hat x1) W (associativity) so the only big exchange is
an AllGather of the dinv-prescaled GAT output y = dinv*x1 in fp8 (26MB),
issued in chunks overlapped with phase-1 compute.  The same one-hot tensor
drives both scatter phases.  Dense GCN (bf16) runs per half-graph interleaved
with phase-2 scatter; graph pooling accumulates in PSUM across tiles.

Pipeline (per core, one NEFF):
  A)  h = x @ W_gat (bf16, replicated), a_src/a_dst folded matmul -> fp8 h_d
  1)  per dst-tile: gather fp8 rows per edge -> logits -> exp ->
      exp*h via DVE+GpSimd split -> DoubleRow one-hot scatter -> y (fp8)
  AG) chunked AllGather of y
  2)  per half: gather y rows, DoubleRow one-hot scatter -> agg; DMA-transpose;
      dense agg @ W_gcn (bf16) with fused relu*dinv; pooling matmul in PSUM
  AR) AllReduce pooled sums, gmean, FC, relu -> out [G, OUT]
"""

import sys
import os
import contextlib

if '/opt/trn_rl_repo' not in sys.path:
    sys.path.insert(0, '/opt/trn_rl_repo')

import numpy as np
import ml_dtypes

import concourse.bacc as bacc
import concourse.mybir as mybir
import concourse.tile as tile
from concourse.bass_utils import run_bass_kernel_spmd

F32 = mybir.dt.float32
BF16 = mybir.dt.bfloat16
F8 = mybir.dt.float8e4
I16 = mybir.dt.int16
BF = ml_dtypes.bfloat16
NPF8 = ml_dtypes.float8_e4m3
Alu = mybir.AluOpType
Act = mybir.ActivationFunctionType
DR = mybir.MatmulPerfMode.DoubleRow


def _ru(x, m):
    return (x + m - 1) // m * m


class Cfg:
    def __init__(self, N, E, H, C, G, OUT, TCT, NCORES=8, GRP=6, HD=26, AGC=1):
        self.N, self.E, self.H, self.C, self.G, self.OUT = N, E, H, C, G, OUT
        self.NCORES = NCORES
        self.D1 = H * C                              # 2496
        self.DP = _ru(self.D1 + 2 * H, 128)          # 2560 fp8 row bytes
        assert self.DP % 256 == 0
        self.NPC = _ru(N, NCORES) // NCORES          # nodes per core
        self.NT = _ru(self.NPC, 128) // 128          # dst tiles per core
        self.XWROWS = self.NT * 128
        self.XWFULL = NCORES * self.XWROWS
        self.ROWS_A = _ru(N, 128) // 128             # stage-A node tiles
        self.NPAD = self.ROWS_A * 128
        self.KS = self.DP // 128                     # dense k slabs
        self.FCK = 2 * self.KS
        assert TCT % 2 == 0
        self.TCT = TCT                               # chunks per dst tile
        self.TC = self.NT * TCT
        self.GRP = GRP                               # chunks per gather group
        assert GRP % 2 == 0
        self.NGRP = (TCT + GRP - 1) // GRP
        self.HD = HD                                 # heads multiplied on DVE
        self.AGC = AGC                               # allgather chunks
        assert self.NT % AGC == 0
        self.TPC = self.NT // AGC                    # tiles per AG chunk
        self.NHALF = 2                               # dense half-phases
        assert self.NT % self.NHALF == 0
        self.HT = self.NT // self.NHALF              # tiles per half
        self.B1NZ = False                            # b_gat nonzero
        self.B2NZ = False                            # b_gcn nonzero


def build(cfg):
    STAGE = int(os.environ.get("GNN_STAGE", "9"))
    DEBUG = int(os.environ.get("GNN_DEBUG", "0"))
    MV = int(os.environ.get("GNN_MV", "0"))
    CD = int(os.environ.get("GNN_CD", "4"))
    hd_env = os.environ.get("GNN_HD")
    if hd_env is not None:
        cfg.HD = int(hd_env)
    c = cfg
    nc = bacc.Bacc(None, target_bir_lowering=False)

    # ---- external inputs (replicated unless noted per-core) ----
    xT = nc.dram_tensor("xT", [c.C, c.NPAD], BF16, kind="ExternalInput")
    Wg = nc.dram_tensor("Wg", [c.C, c.D1], BF16, kind="ExternalInput")
    Mcat = nc.dram_tensor("Mcat", [c.C, 2 * c.H], BF16, kind="ExternalInput")
    Wgcn = nc.dram_tensor("Wgcn", [c.DP, c.DP], BF16, kind="ExternalInput")
    Wfc = nc.dram_tensor("Wfc", [2 * c.DP, c.OUT], F32, kind="ExternalInput")
    bfc = nc.dram_tensor("bfc", [c.G, c.OUT], F32, kind="ExternalInput")
    bgat = nc.dram_tensor("bgat", [128, c.DP], F32, kind="ExternalInput")
    bgcn = nc.dram_tensor("bgcn", [128, c.DP], F32, kind="ExternalInput")
    invcnt = nc.dram_tensor("invcnt", [128, c.G], F32, kind="ExternalInput")
    # per-core:
    sidx = nc.dram_tensor("sidx", [128, c.TC * 8], I16, kind="ExternalInput")
    yidx = nc.dram_tensor("yidx", [128, c.TC * 8], I16, kind="ExternalInput")
    dnid = nc.dram_tensor("dnid", [128, c.NT * 8], I16, kind="ExternalInput")
    ohb1 = nc.dram_tensor("ohb1", [128, c.TC, 128], BF16, kind="ExternalInput")
    ohb2 = nc.dram_tensor("ohb2", [128, c.TC, 128], F8, kind="ExternalInput")
    ohT = nc.dram_tensor("ohT", [128, c.TC, 128], BF16, kind="ExternalInput")
    scl = nc.dram_tensor("scl", [128, c.NT], F32, kind="ExternalInput")
    gon = nc.dram_tensor("gon", [128, c.NT, c.G], BF16, kind="ExternalInput")
    out = nc.dram_tensor("out", [c.G, c.OUT], F32, kind="ExternalOutput")
    if DEBUG:
        dbg_h = nc.dram_tensor("dbg_h", [c.NPAD, c.DP], BF16,
                               kind="ExternalOutput")
        dbg_y = nc.dram_tensor("dbg_y", [c.XWFULL, c.DP], mybir.dt.uint8,
                               kind="ExternalOutput")
        dbg_a = nc.dram_tensor("dbg_a", [c.XWROWS, c.DP], BF16,
                               kind="ExternalOutput")
        dbg_g = nc.dram_tensor("dbg_g", [128, c.KS * c.G], F32,
                               kind="ExternalOutput")
        dbg_x2 = nc.dram_tensor("dbg_x2", [c.XWROWS, c.DP], BF16,
                                kind="ExternalOutput")

    rg = [list(range(c.NCORES))]

    with tile.TileContext(nc) as tc:
        with (
            tc.tile_pool(name="dram", bufs=1, space="DRAM") as dram,
            tc.tile_pool(name="persist", bufs=1) as pp,
        ):
            h_d = dram.tile([c.NPAD, c.DP], BF16)
            y_d = dram.tile([c.XWROWS, c.DP], F8)
            yf_d = dram.tile([c.XWFULL, c.DP], F8, addr_space="Shared")
            aggb_d = dram.tile([c.XWROWS, c.DP], BF16)
            gs_in_d = dram.tile([128, c.KS * c.G], F32)
            gs_out_d = dram.tile([128, c.KS * c.G], F32, addr_space="Shared")

            # persistent smalls + resident GCN weights
            scl_sb = pp.tile([128, c.NT], F32)
            nc.sync.dma_start(scl_sb[:], scl[:])
            c02 = pp.tile([128, 1], BF16)
            nc.vector.memset(c02[:], 0.2)
            gon_sb = pp.tile([128, c.NT, c.G], BF16)
            nc.sync.dma_start(gon_sb[:], gon[:])
            wgcn_sb = pp.tile([128, c.KS, c.DP], BF16)
            for k in range(c.KS):
                nc.sync.dma_start(wgcn_sb[:, k, :],
                                  Wgcn[k * 128:(k + 1) * 128, :])
            if c.B1NZ:
                bgat_sb = pp.tile([128, c.DP], F32)
                nc.sync.dma_start(bgat_sb[:], bgat[:])
            if c.B2NZ:
                bgcn_sb = pp.tile([128, c.DP], F32)
                nc.sync.dma_start(bgcn_sb[:], bgcn[:])

            # ============ Stage A: h = x@Wg -> fp8 h_d with a-tail ============
            with tc.tile_pool(name="stageA", bufs=2) as sa, \
                 tc.tile_pool(name="stageAc", bufs=1) as sac, \
                 tc.tile_pool(name="psH", bufs=3, space="PSUM") as psH, \
                 tc.tile_pool(name="psHa", bufs=2, space="PSUM") as psHa:
                xT_sb = sac.tile([c.C, c.NPAD], BF16)
                nc.sync.dma_start(xT_sb[:], xT[:])
                Wg_sb = sac.tile([c.C, c.D1], BF16)
                nc.sync.dma_start(Wg_sb[:], Wg[:])
                Mc_sb = sac.tile([c.C, 2 * c.H], BF16)
                nc.sync.dma_start(Mc_sb[:], Mcat[:])
                for r in range(c.ROWS_A if STAGE >= 1 else 0):
                    lhs = xT_sb[:, r * 128:(r + 1) * 128]
                    hb = sa.tile([128, c.DP], BF16, tag="hb")
                    for i, j0 in enumerate(range(0, c.D1, 512)):
                        j1 = min(j0 + 512, c.D1)
                        ph = psH.tile([128, 512], F32, tag="ph")
                        nc.tensor.matmul(ph[:, 0:j1 - j0], lhs, Wg_sb[:, j0:j1],
                                         start=True, stop=True)
                        if i % 2 == 0:
                            nc.scalar.copy(hb[:, j0:j1], ph[:, 0:j1 - j0])
                        else:
                            nc.vector.tensor_copy(hb[:, j0:j1], ph[:, 0:j1 - j0])
                    pa = psHa.tile([128, 2 * c.H], F32, tag="pa")
                    nc.tensor.matmul(pa[:], lhs, Mc_sb[:], start=True, stop=True)
                    nc.vector.tensor_copy(hb[:, c.D1:c.D1 + 2 * c.H], pa[:])
                    nc.sync.dma_start(h_d[r * 128:(r + 1) * 128, :], hb[:])

            psA = contextlib.ExitStack()
            psA_pool = psA.enter_context(
                tc.tile_pool(name="psA", bufs=1, space="PSUM"))

            # ============ Phase 1: GAT edge softmax + scatter -> y ============
            with tc.tile_pool(name="p1", bufs=2) as p1, \
                 tc.tile_pool(name="p1h", bufs=2) as p1h, \
                 tc.tile_pool(name="p1o", bufs=2) as p1o, \
                 tc.tile_pool(name="psD", bufs=1, space="PSUM") as psD, \
                 tc.tile_pool(name="psE", bufs=2, space="PSUM") as psE:
                for t in range(c.NT if STAGE >= 2 else 0):
                    cs = t * c.TCT * 8
                    ce = (t + 1) * c.TCT * 8
                    si = p1.tile([128, c.TCT * 8], I16, tag="si")
                    nc.sync.dma_start(si[:], sidx[:, cs:ce])
                    ob = p1o.tile([128, c.TCT, 128], F8, tag="ob")
                    nc.sync.dma_start(ob[:], ohb2[:, t * c.TCT:(t + 1) * c.TCT, :])
                    obw = p1o.tile([128, c.TCT, 128], BF16, tag="obw")
                    nc.sync.dma_start(obw[:], ohb1[:, t * c.TCT:(t + 1) * c.TCT, :])
                    oT = p1o.tile([128, c.TCT, 128], BF16, tag="oT")
                    nc.sync.dma_start(oT[:], ohT[:, t * c.TCT:(t + 1) * c.TCT, :])
                    dn = p1.tile([128, 8], I16, tag="dn")
                    nc.sync.dma_start(dn[:], dnid[:, t * 8:(t + 1) * 8])
                    adt = p1.tile([128, 1, 128], BF16, tag="adt")
                    nc.gpsimd.dma_gather(adt[:], h_d[:, c.DP - 128:c.DP],
                                         dn[:], 128, 128, 128, elem_step=c.DP)

                    px = psA_pool.tile([128, c.DP], F32, tag="px")
                    pd = psD.tile([128, c.H], F32, tag="pd")
                    exf = p1.tile([128, c.TCT, c.H], BF16, tag="exf")
                    exf2 = p1.tile([128, c.TCT, c.H], BF16, tag="exf2")
                    ex8 = p1.tile([128, c.TCT, c.H], F8, tag="ex8")
                    def p1_gather(g):
                        c0 = g * c.GRP
                        c1 = min(c0 + c.GRP, c.TCT)
                        nch = c1 - c0
                        hgt = p1h.tile([128, c.GRP, c.DP], BF16, tag="hg")
                        nc.gpsimd.dma_gather(hgt[:, 0:nch, :], h_d[:],
                                             si[:, c0 * 8:c1 * 8],
                                             nch * 128, nch * 128, c.DP)
                        return hgt

                    hgs = {0: p1_gather(0)}
                    for g in range(c.NGRP):
                        c0 = g * c.GRP
                        c1 = min(c0 + c.GRP, c.TCT)
                        nch = c1 - c0
                        if g + 1 < c.NGRP:
                            hgs[g + 1] = p1_gather(g + 1)
                        hg = hgs.pop(g)
                        peg = psE.tile([128, c.GRP, c.H], F32, tag="peg")
                        for ch in range(c0, c1):
                            nc.tensor.matmul(
                                peg[:, ch - c0, :], oT[:, ch, :],
                                adt[:, 0, 128 - c.H:128],
                                start=True, stop=True)
                        ev = exf[:, c0:c1, :]
                        nc.vector.tensor_add(ev, peg[:, 0:nch, :],
                                             hg[:, 0:nch, c.D1:c.D1 + c.H])
                        ev2 = exf2[:, c0:c1, :]
                        nc.vector.tensor_tensor(
                            ev2, ev,
                            c02[:, :, None].broadcast_to([128, nch, c.H]),
                            Alu.mult)
                        nc.vector.tensor_tensor(ev, ev, ev2, Alu.max)
                        nc.scalar.activation(ev, ev, Act.Exp)
                        nc.scalar.copy(ex8[:, c0:c1, :], ev)
                        mv = hg[:, 0:nch, 0:c.D1].rearrange(
                            "p t (h w) -> p t h w", h=c.H)
                        ebl = exf[:, c0:c1, 0:c.HD, None].broadcast_to(
                            [128, nch, c.HD, c.C])
                        ebp = exf[:, c0:c1, c.HD:c.H, None].broadcast_to(
                            [128, nch, c.H - c.HD, c.C])
                        nc.vector.tensor_mul(mv[:, :, 0:c.HD, :],
                                             mv[:, :, 0:c.HD, :], ebl)
                        if c.HD < c.H:
                            nc.gpsimd.tensor_mul(mv[:, :, c.HD:c.H, :],
                                                 mv[:, :, c.HD:c.H, :], ebp)
                        for ch2 in range(c0, c1, 2):
                            first = (ch2 == 0)
                            last = (ch2 == c.TCT - 2)
                            nc.tensor.matmul(pd[:], ob[:, ch2:ch2 + 2, :],
                                             ex8[:, ch2:ch2 + 2, :],
                                             start=first, stop=last,
                                             perf_mode=DR)
                        for ch in range(c0, c1):
                            for j0 in range(0, c.DP, 512):
                                nc.tensor.matmul(
                                    px[:, j0:j0 + 512], obw[:, ch, :],
                                    hg[:, ch - c0, j0:j0 + 512],
                                    start=(ch == 0), stop=(ch == c.TCT - 1))
                    rdn = p1.tile([128, c.H], F32, tag="rdn")
                    nc.vector.reciprocal(rdn[:], pd[:])
                    sc = p1.tile([128, c.H], F32, tag="sc")
                    nc.vector.tensor_mul(
                        sc[:], rdn[:],
                        scl_sb[:, t:t + 1].broadcast_to([128, c.H]))
                    yt = p1.tile([128, c.DP], F8, tag="yt")
                    nc.vector.memset(yt[:, c.D1:], 0.0)
                    pxv = px[:, 0:c.D1].rearrange("p (h w) -> p h w", h=c.H)
                    ytv = yt[:, 0:c.D1].rearrange("p (h w) -> p h w", h=c.H)
                    scb = sc[:, :, None].broadcast_to([128, c.H, c.C])
                    if not c.B1NZ:
                        nc.vector.scalar_tensor_tensor(ytv, pxv, 0.0, scb,
                                                       Alu.max, Alu.mult)
                    else:
                        x1f = p1.tile([128, c.D1], F32, tag="x1f")
                        x1v = x1f[:].rearrange("p (h w) -> p h w", h=c.H)
                        rb = rdn[:, :, None].broadcast_to([128, c.H, c.C])
                        nc.vector.tensor_mul(x1v, pxv, rb)
                        nc.vector.tensor_add(x1f[:], x1f[:],
                                             bgat_sb[:, 0:c.D1])
                        nc.vector.tensor_scalar_max(x1f[:], x1f[:], 0.0)
                        dvb = scl_sb[:, t:t + 1].broadcast_to([128, c.D1])
                        nc.vector.tensor_tensor(yt[:, 0:c.D1], x1f[:], dvb,
                                                Alu.mult)
                    nc.sync.dma_start(y_d[t * 128:(t + 1) * 128, :], yt[:])
                    # chunked AllGather as soon as a chunk's tiles are done
                    if STAGE >= 3 and (t + 1) % c.TPC == 0:
                        k = (t + 1) // c.TPC - 1
                        r0 = k * c.TPC * 128
                        r1 = (k + 1) * c.TPC * 128
                        nc.gpsimd.collective_compute(
                            "AllGather", Alu.bypass,
                            ins=[y_d[r0:r1, :]],
                            outs=[yf_d[r0 * c.NCORES:r1 * c.NCORES, :]],
                            replica_groups=rg)

            if DEBUG:
                nc.sync.dma_start(dbg_h[:], h_d[:])
                nc.sync.dma_start(dbg_y[:], yf_d[:].bitcast(mybir.dt.uint8))

            # ============ Phase 2: GCN scatter + dense + pooling ============
            with tc.tile_pool(name="p2", bufs=2) as p2, \
                 tc.tile_pool(name="p2h", bufs=2) as p2h, \
                 tc.tile_pool(name="p2o", bufs=2) as p2o, \
                 tc.tile_pool(name="gd", bufs=1) as gd, \
                 tc.tile_pool(name="gw", bufs=2) as gw, \
                 tc.tile_pool(name="psW", bufs=2, space="PSUM") as psW, \
                 tc.tile_pool(name="psP", bufs=1, space="PSUM") as psP:
                gacc = pp.tile([128, c.KS * c.G], F32)
                nc.vector.memset(gacc[:], 0.0)

                for hf in range(c.NHALF if STAGE >= 4 else 0):
                    for t in range(hf * c.HT, (hf + 1) * c.HT):
                        cs = t * c.TCT * 8
                        ce = (t + 1) * c.TCT * 8
                        xi = p2.tile([128, c.TCT * 8], I16, tag="xi")
                        nc.sync.dma_start(xi[:], yidx[:, cs:ce])
                        ob2 = p2o.tile([128, c.TCT, 128], F8, tag="ob2")
                        nc.sync.dma_start(ob2[:],
                                          ohb2[:, t * c.TCT:(t + 1) * c.TCT, :])
                        px2 = psA_pool.tile([128, c.DP], F32, tag="px")
                        for g in range(c.NGRP):
                            c0 = g * c.GRP
                            c1 = min(c0 + c.GRP, c.TCT)
                            nch = c1 - c0
                            yg = p2h.tile([128, c.GRP, c.DP], F8, tag="hg")
                            nc.gpsimd.dma_gather(yg[:, 0:nch, :], yf_d[:],
                                                 xi[:, c0 * 8:c1 * 8],
                                                 nch * 128, nch * 128, c.DP)
                            for ch2 in range(c0, c1, 2):
                                first = (ch2 == 0)
                                last = (ch2 == c.TCT - 2)
                                for j0 in range(0, c.DP, 512):
                                    nc.tensor.matmul(
                                        px2[:, j0:j0 + 512],
                                        ob2[:, ch2:ch2 + 2, :],
                                        yg[:, ch2 - c0:ch2 - c0 + 2,
                                           j0:j0 + 512],
                                        start=first, stop=last, perf_mode=DR)
                        agt = p2.tile([128, c.DP], BF16, tag="agt")
                        nc.scalar.copy(agt[:], px2[:])
                        nc.sync.dma_start(aggb_d[t * 128:(t + 1) * 128, :],
                                          agt[:])
                    if STAGE < 5:
                        continue
                    # dense for this half: transpose agg, matmul, relu*dinv
                    hr0 = hf * c.HT * 128
                    hr1 = (hf + 1) * c.HT * 128
                    aggT = gd.tile([128, c.KS, c.HT * 128], BF16, tag="aT")
                    for k in range(c.KS):
                        nc.sync.dma_start(aggT[:, k, :],
                                          aggb_d[hr0:hr1, k * 128:(k + 1) * 128],
                                          transpose=True)
                    for m in range(hf * c.HT, (hf + 1) * c.HT):
                        mo = (m - hf * c.HT) * 128
                        xt2 = gw.tile([128, c.DP], BF16, tag="xt2")
                        for j0 in range(0, c.DP, 512):
                            pw = psW.tile([128, 512], F32, tag="pw")
                            for k in range(c.KS):
                                nc.tensor.matmul(
                                    pw[:],
                                    aggT[:, k, mo:mo + 128],
                                    wgcn_sb[:, k, j0:j0 + 512],
                                    start=(k == 0), stop=(k == c.KS - 1))
                            if not c.B2NZ:
                                nc.scalar.activation(
                                    xt2[:, j0:j0 + 512], pw[:],
                                    Act.Relu, scale=scl_sb[:, m:m + 1])
                            else:
                                xf = gw.tile([128, 512], F32, tag="xf")
                                dvb = scl_sb[:, m:m + 1].broadcast_to(
                                    [128, 512])
                                nc.vector.tensor_tensor(
                                    xf[:], pw[:], dvb, Alu.mult)
                                nc.vector.tensor_add(
                                    xf[:], xf[:], bgcn_sb[:, j0:j0 + 512])
                                nc.vector.tensor_scalar_max(
                                    xt2[:, j0:j0 + 512], xf[:], 0.0)
                        if DEBUG:
                            nc.sync.dma_start(
                                dbg_x2[m * 128:(m + 1) * 128, :], xt2[:])
                        for fb in range(0, c.KS, 8):
                            fe = min(fb + 8, c.KS)
                            pgt = psP.tile([128, 8, c.G], F32, tag="pg")
                            for fs in range(fb, fe):
                                nc.tensor.matmul(
                                    pgt[:, fs - fb, :],
                                    xt2[:, fs * 128:(fs + 1) * 128],
                                    gon_sb[:, m, :],
                                    start=True, stop=True)
                            nc.vector.tensor_add(
                                gacc[:, fb * c.G:fe * c.G],
                                gacc[:, fb * c.G:fe * c.G],
                                pgt[:, 0:fe - fb, :].rearrange(
                                    "p k g -> p (k g)"))
                nc.gpsimd.dma_start(gs_in_d[:], gacc[:])
                if DEBUG:
                    nc.sync.dma_start(dbg_a[:], aggb_d[:])
                    nc.sync.dma_start(dbg_g[:], gs_in_d[:])
            psA.close()

            # ============ AllReduce pooled sums + FC ============
            if STAGE >= 6:
                nc.gpsimd.collective_compute(
                    "AllReduce", Alu.add, ins=[gs_in_d[:]], outs=[gs_out_d[:]],
                    replica_groups=rg)
            with tc.tile_pool(name="fc", bufs=1) as fc, \
                 tc.tile_pool(name="psS", bufs=1, space="PSUM") as psS:
              if STAGE < 6:
                dz = fc.tile([c.G, c.OUT], F32)
                nc.vector.memset(dz[:], 0.0)
                nc.sync.dma_start(out[:], dz[:])
              else:
                gsar = fc.tile([128, c.KS, c.G], F32)
                nc.sync.dma_start(gsar[:],
                                  gs_out_d[:].rearrange("p (k g) -> p k g",
                                                        k=c.KS))
                iv_sb = fc.tile([128, c.G], F32)
                nc.sync.dma_start(iv_sb[:], invcnt[:])
                gm = fc.tile([128, c.KS, c.G], F32)
                nc.vector.tensor_mul(
                    gm[:], gsar[:],
                    iv_sb[:, None, :].broadcast_to([128, c.KS, c.G]))
                wf_sb = fc.tile([128, c.FCK, c.OUT], F32)
                nc.sync.dma_start(
                    wf_sb[:], Wfc[:].rearrange("(k p) o -> p k o", p=128))
                pf = psS.tile([c.G, c.OUT], F32, tag="sm")
                for k in range(c.FCK):
                    lhs = gm[:, k, :] if k < c.KS else gsar[:, k - c.KS, :]
                    nc.tensor.matmul(pf[:], lhs, wf_sb[:, k, :],
                                     start=(k == 0), stop=(k == c.FCK - 1))
                bf_sb = fc.tile([c.G, c.OUT], F32)
                nc.sync.dma_start(bf_sb[:], bfc[:])
                ot = fc.tile([c.G, c.OUT], F32)
                nc.vector.tensor_add(ot[:], pf[:], bf_sb[:])
                nc.vector.tensor_scalar_max(ot[:], ot[:], 0.0)
                nc.sync.dma_start(out[:], ot[:])

    nc.compile()
    return nc


# ================= host-side preprocessing =================

def _wrap_idx(a):
    """[L] int -> [128, L//16] int16 wrapped (i -> [i%16, i//16]) + 8x repl."""
    w = a.reshape(-1, 16).T.astype(np.int16)
    return np.tile(w, (8, 1)).copy()


def preprocess(x, edge_index, batch, num_graphs, W_gat, att_src, att_dst,
               b_gat, W_gcn, b_gcn, W_fc, b_fc, cfg=None, ncores=8):
    N, C = x.shape
    E = edge_index.shape[1]
    H = att_src.shape[0]
    G = int(num_graphs)
    OUT = W_fc.shape[1]

    src = np.concatenate([np.asarray(edge_index[0]), np.arange(N)]).astype(np.int64)
    dst = np.concatenate([np.asarray(edge_index[1]), np.arange(N)]).astype(np.int64)
    deg = np.bincount(dst, minlength=N).astype(np.float32)
    dinv = np.where(deg > 0, 1.0 / np.sqrt(deg), 0.0).astype(np.float32)

    NC_ = ncores
    NPC = _ru(N, NC_) // NC_
    NT = _ru(NPC, 128) // 128

    order = np.argsort(dst, kind='stable')
    s_s, s_d = src[order], dst[order]

    # per (core,tile) edge lists
    tiles = [[None] * NT for _ in range(NC_)]
    for core in range(NC_):
        for t in range(NT):
            lo = np.searchsorted(s_d, core * NPC + t * 128)
            hi = np.searchsorted(s_d, min(core * NPC + (t + 1) * 128,
                                          (core + 1) * NPC))
            tiles[core][t] = (s_s[lo:hi], s_d[lo:hi])

    TCT = max(max(_ru(len(tt[0]), 128) // 128 for tt in row) for row in tiles)
    TCT = max(_ru(TCT, 2), 2)
    if cfg is None:
        cfg = Cfg(N, E, H, C, G, OUT, TCT, NCORES=NC_)
        cfg.B1NZ = bool(np.any(np.asarray(b_gat) != 0))
        cfg.B2NZ = bool(np.any(np.asarray(b_gcn) != 0))
    assert cfg.TCT == TCT

    c = cfg
    # replicated tensors
    xT = np.zeros((C, c.NPAD), BF)
    xT[:, :N] = np.asarray(x).T.astype(BF)
    Wgf = np.asarray(W_gat).astype(np.float32)
    Wg = Wgf.astype(BF)
    Wg3 = Wgf.reshape(C, H, C)
    Mcat = np.zeros((C, 2 * H), BF)
    Mcat[:, 0:H] = np.einsum('khc,hc->kh', Wg3, np.asarray(att_src)).astype(BF)
    Mcat[:, H:2 * H] = np.einsum('khc,hc->kh', Wg3, np.asarray(att_dst)).astype(BF)
    bgat = np.zeros((128, c.DP), np.float32)
    bgat[:, :c.D1] = np.asarray(b_gat)[None, :]
    bgcn = np.zeros((128, c.DP), np.float32)
    bgcn[:, :c.D1] = np.asarray(b_gcn)[None, :]
    Wgcn = np.zeros((c.DP, c.DP), BF)
    Wgcn[:c.D1, :c.D1] = np.asarray(W_gcn).astype(BF)
    Wfc = np.zeros((2 * c.DP, OUT), np.float32)
    Wfc[0:c.D1] = np.asarray(W_fc)[0:c.D1]
    Wfc[c.DP:c.DP + c.D1] = np.asarray(W_fc)[c.D1:2 * c.D1]
    bfc = np.tile(np.asarray(b_fc).astype(np.float32)[None, :], (G, 1))
    cnt = np.bincount(np.asarray(batch), minlength=G).astype(np.float32)
    invcnt = np.tile((1.0 / np.maximum(cnt, 1.0))[None, :], (128, 1))

    batch_np = np.asarray(batch)
    shared = dict(xT=xT, Wg=Wg, Mcat=Mcat, Wgcn=Wgcn, Wfc=Wfc, bfc=bfc,
                  invcnt=invcnt, bgat=bgat, bgcn=bgcn)

    # y row index in the chunk-wise AllGathered layout, per source node id
    def yrow_of(j, core_of):
        local = j - core_of * NPC
        t = local // 128
        r = local % 128
        k = t // c.TPC
        return (k * c.NCORES * c.TPC * 128 + core_of * c.TPC * 128
                + (t - k * c.TPC) * 128 + r)

    in_maps = []
    for core in range(NC_):
        L = c.TC * 128
        sp = np.zeros(L, np.int64)
        dl = np.zeros(L, np.int64)
        valid = np.zeros(L, bool)
        for t in range(NT):
            ts, td = tiles[core][t]
            o = t * c.TCT * 128
            k = len(ts)
            sp[o:o + k] = ts
            dl[o:o + k] = td - (core * NPC + t * 128)
            valid[o:o + k] = True
        cs = sp // NPC
        yr = np.array([yrow_of(j, cj) for j, cj in zip(sp, cs)], np.int64)
        oh = np.zeros((c.TC, 128, 128), np.float32)
        ee = np.arange(L)
        oh[ee // 128, ee % 128, dl] = valid.astype(np.float32)
        # this core's dst-node ids per (tile, slot), clamped to valid rows
        dnids = np.zeros((NT, 128), np.int64)
        for t in range(NT):
            gids = core * NPC + t * 128 + np.arange(128)
            dnids[t] = np.minimum(gids, N - 1)
        sclm = np.zeros((128, NT), np.float32)
        for t in range(NT):
            gids = core * NPC + t * 128 + np.arange(128)
            ok = gids < min((core + 1) * NPC, N)
            sclm[ok, t] = dinv[gids[ok]]
        gonm = np.zeros((128, NT, G), BF)
        for t in range(NT):
            gids = core * NPC + t * 128 + np.arange(128)
            ok = gids < min((core + 1) * NPC, N)
            gonm[ok, t, batch_np[gids[ok]]] = 1.0
        m = dict(shared)
        m.update(
            sidx=_wrap_idx(sp), yidx=_wrap_idx(yr),
            dnid=_wrap_idx(dnids.reshape(-1)),
            ohb1=oh.transpose(1, 0, 2).astype(BF),
            ohb2=oh.transpose(1, 0, 2).astype(NPF8),
            ohT=oh.transpose(2, 0, 1).astype(BF),
            scl=sclm, gon=gonm)
        in_maps.append(m)
    return cfg, in_maps


_CACHE = {}


def run(inputs, trace=False):
    key = tuple(sorted((k, tuple(np.shape(v))) for k, v in inputs.items()))
    cfg, in_maps = preprocess(**inputs,
                              cfg=_CACHE[key][0] if key in _CACHE else None)
    if key not in _CACHE:
        _CACHE[key] = (cfg, build(cfg))
    cfg, nc = _CACHE[key]
    res = run_bass_kernel_spmd(nc, in_maps, core_ids=list(range(cfg.NCORES)),
                               trace=trace)
    return res.results[0]["out"].astype(np.float32), res


def kernel(**inputs):
    out, _ = run(inputs)
    return out


# revision 45
# speedup vs baseline: 1.2073x; 1.2073x over previous
"""GAT+GCN+pool GNN on 8 Trainium2 NeuronCores (Bass/Tile), fp8 edition.

Sharding: nodes/edges partitioned across 8 cores by destination-node range;
segment softmax and scatter-adds are core-local.  Per-edge row gathers use
dma_gather on fp8 rows (h stored as [2496 h | 32 a_src | 32 a_dst] fp8e4),
scatter-adds are DoubleRow fp8 one-hot matmuls (256 edges per pass).

GCN is computed as (A_hat x1) W (associativity) so the only big exchange is
an AllGather of the dinv-prescaled GAT output y = dinv*x1 in fp8 (26MB),
issued in chunks overlapped with phase-1 compute.  The same one-hot tensor
drives both scatter phases.  Dense GCN (bf16) runs per half-graph interleaved
with phase-2 scatter; graph pooling accumulates in PSUM across tiles.

Pipeline (per core, one NEFF):
  A)  h = x @ W_gat (bf16, replicated), a_src/a_dst folded matmul -> fp8 h_d
  1)  per dst-tile: gather fp8 rows per edge -> logits -> exp ->
      exp*h via DVE+GpSimd split -> DoubleRow one-hot scatter -> y (fp8)
  AG) chunked AllGather of y
  2)  per half: gather y rows, DoubleRow one-hot scatter -> agg; DMA-transpose;
      dense agg @ W_gcn (bf16) with fused relu*dinv; pooling matmul in PSUM
  AR) AllReduce pooled sums, gmean, FC, relu -> out [G, OUT]
"""

import sys
import os
import contextlib

if '/opt/trn_rl_repo' not in sys.path:
    sys.path.insert(0, '/opt/trn_rl_repo')

import numpy as np
import ml_dtypes

import concourse.bacc as bacc
import concourse.mybir as mybir
import concourse.tile as tile
from concourse.bass_utils import run_bass_kernel_spmd

F32 = mybir.dt.float32
BF16 = mybir.dt.bfloat16
F8 = mybir.dt.float8e4
I16 = mybir.dt.int16
BF = ml_dtypes.bfloat16
NPF8 = ml_dtypes.float8_e4m3
Alu = mybir.AluOpType
Act = mybir.ActivationFunctionType
DR = mybir.MatmulPerfMode.DoubleRow


def _ru(x, m):
    return (x + m - 1) // m * m


class Cfg:
    def __init__(self, N, E, H, C, G, OUT, TCT, NCORES=8, GRP=6, HD=22, AGC=1):
        self.N, self.E, self.H, self.C, self.G, self.OUT = N, E, H, C, G, OUT
        self.NCORES = NCORES
        self.D1 = H * C                              # 2496
        self.DP = _ru(self.D1 + 2 * H, 128)          # 2560 fp8 row bytes
        assert self.DP % 256 == 0
        self.NPC = _ru(N, NCORES) // NCORES          # nodes per core
        self.NT = _ru(self.NPC, 128) // 128          # dst tiles per core
        self.XWROWS = self.NT * 128
        self.XWFULL = NCORES * self.XWROWS
        self.ROWS_A = _ru(N, 128) // 128             # stage-A node tiles
        self.NPAD = self.ROWS_A * 128
        self.KS = self.DP // 128                     # dense k slabs
        self.FCK = 2 * self.KS
        assert TCT % 2 == 0
        self.TCT = TCT                               # chunks per dst tile
        self.TC = self.NT * TCT
        self.GRP = GRP                               # chunks per gather group
        assert GRP % 2 == 0
        self.NGRP = (TCT + GRP - 1) // GRP
        self.HD = HD                                 # heads multiplied on DVE
        self.AGC = AGC                               # allgather chunks
        assert self.NT % AGC == 0
        self.TPC = self.NT // AGC                    # tiles per AG chunk
        self.NHALF = 2                               # dense half-phases
        assert self.NT % self.NHALF == 0
        self.HT = self.NT // self.NHALF              # tiles per half
        self.B1NZ = False                            # b_gat nonzero
        self.B2NZ = False                            # b_gcn nonzero


def build(cfg):
    STAGE = int(os.environ.get("GNN_STAGE", "9"))
    DEBUG = int(os.environ.get("GNN_DEBUG", "0"))
    MV = int(os.environ.get("GNN_MV", "0"))
    CD = int(os.environ.get("GNN_CD", "4"))
    hd_env = os.environ.get("GNN_HD")
    if hd_env is not None:
        cfg.HD = int(hd_env)
    c = cfg
    nc = bacc.Bacc(None, target_bir_lowering=False)

    # ---- external inputs (replicated unless noted per-core) ----
    xT = nc.dram_tensor("xT", [c.C, c.NPAD], BF16, kind="ExternalInput")
    Wg = nc.dram_tensor("Wg", [c.C, c.D1], BF16, kind="ExternalInput")
    Mcat = nc.dram_tensor("Mcat", [c.C, 2 * c.H], BF16, kind="ExternalInput")
    Wgcn = nc.dram_tensor("Wgcn", [c.DP, c.DP], BF16, kind="ExternalInput")
    Wfc = nc.dram_tensor("Wfc", [2 * c.DP, c.OUT], F32, kind="ExternalInput")
    bfc = nc.dram_tensor("bfc", [c.G, c.OUT], F32, kind="ExternalInput")
    bgat = nc.dram_tensor("bgat", [128, c.DP], F32, kind="ExternalInput")
    bgcn = nc.dram_tensor("bgcn", [128, c.DP], F32, kind="ExternalInput")
    invcnt = nc.dram_tensor("invcnt", [128, c.G], F32, kind="ExternalInput")
    # per-core:
    sidx = nc.dram_tensor("sidx", [128, c.TC * 8], I16, kind="ExternalInput")
    yidx = nc.dram_tensor("yidx", [128, c.TC * 8], I16, kind="ExternalInput")
    dnid = nc.dram_tensor("dnid", [128, c.NT * 8], I16, kind="ExternalInput")
    ohb1 = nc.dram_tensor("ohb1", [128, c.TC, 128], BF16, kind="ExternalInput")
    ohb2 = nc.dram_tensor("ohb2", [128, c.TC, 128], F8, kind="ExternalInput")
    ohT = nc.dram_tensor("ohT", [128, c.TC, 128], BF16, kind="ExternalInput")
    scl = nc.dram_tensor("scl", [128, c.NT], F32, kind="ExternalInput")
    gon = nc.dram_tensor("gon", [128, c.NT, c.G], BF16, kind="ExternalInput")
    out = nc.dram_tensor("out", [c.G, c.OUT], F32, kind="ExternalOutput")
    if DEBUG:
        dbg_h = nc.dram_tensor("dbg_h", [c.NPAD, c.DP], BF16,
                               kind="ExternalOutput")
        dbg_y = nc.dram_tensor("dbg_y", [c.XWFULL, c.DP], mybir.dt.uint8,
                               kind="ExternalOutput")
        dbg_a = nc.dram_tensor("dbg_a", [c.XWROWS, c.DP], BF16,
                               kind="ExternalOutput")
        dbg_g = nc.dram_tensor("dbg_g", [128, c.KS * c.G], F32,
                               kind="ExternalOutput")
        dbg_x2 = nc.dram_tensor("dbg_x2", [c.XWROWS, c.DP], BF16,
                                kind="ExternalOutput")

    rg = [list(range(c.NCORES))]

    with tile.TileContext(nc) as tc:
        with (
            tc.tile_pool(name="dram", bufs=1, space="DRAM") as dram,
            tc.tile_pool(name="persist", bufs=1) as pp,
        ):
            h_d = dram.tile([c.NPAD, c.DP], BF16)
            y_d = dram.tile([c.XWROWS, c.DP], F8)
            yf_d = dram.tile([c.XWFULL, c.DP], F8, addr_space="Shared")
            aggb_d = dram.tile([c.XWROWS, c.DP], BF16)
            gs_in_d = dram.tile([128, c.KS * c.G], F32)
            gs_out_d = dram.tile([128, c.KS * c.G], F32, addr_space="Shared")

            # persistent smalls + resident GCN weights
            scl_sb = pp.tile([128, c.NT], F32)
            nc.sync.dma_start(scl_sb[:], scl[:])
            c02 = pp.tile([128, 1], BF16)
            nc.vector.memset(c02[:], 0.2)
            gon_sb = pp.tile([128, c.NT, c.G], BF16)
            nc.sync.dma_start(gon_sb[:], gon[:])
            wgcn_sb = pp.tile([128, c.KS, c.DP], BF16)
            for k in range(c.KS):
                nc.sync.dma_start(wgcn_sb[:, k, :],
                                  Wgcn[k * 128:(k + 1) * 128, :])
            if c.B1NZ:
                bgat_sb = pp.tile([128, c.DP], F32)
                nc.sync.dma_start(bgat_sb[:], bgat[:])
            if c.B2NZ:
                bgcn_sb = pp.tile([128, c.DP], F32)
                nc.sync.dma_start(bgcn_sb[:], bgcn[:])

            # ============ Stage A: h = x@Wg -> fp8 h_d with a-tail ============
            with tc.tile_pool(name="stageA", bufs=2) as sa, \
                 tc.tile_pool(name="stageAc", bufs=1) as sac, \
                 tc.tile_pool(name="psH", bufs=3, space="PSUM") as psH, \
                 tc.tile_pool(name="psHa", bufs=2, space="PSUM") as psHa:
                xT_sb = sac.tile([c.C, c.NPAD], BF16)
                nc.sync.dma_start(xT_sb[:], xT[:])
                Wg_sb = sac.tile([c.C, c.D1], BF16)
                nc.sync.dma_start(Wg_sb[:], Wg[:])
                Mc_sb = sac.tile([c.C, 2 * c.H], BF16)
                nc.sync.dma_start(Mc_sb[:], Mcat[:])
                for r in range(c.ROWS_A if STAGE >= 1 else 0):
                    lhs = xT_sb[:, r * 128:(r + 1) * 128]
                    hb = sa.tile([128, c.DP], BF16, tag="hb")
                    for i, j0 in enumerate(range(0, c.D1, 512)):
                        j1 = min(j0 + 512, c.D1)
                        ph = psH.tile([128, 512], F32, tag="ph")
                        nc.tensor.matmul(ph[:, 0:j1 - j0], lhs, Wg_sb[:, j0:j1],
                                         start=True, stop=True)
                        if i % 2 == 0:
                            nc.scalar.copy(hb[:, j0:j1], ph[:, 0:j1 - j0])
                        else:
                            nc.vector.tensor_copy(hb[:, j0:j1], ph[:, 0:j1 - j0])
                    pa = psHa.tile([128, 2 * c.H], F32, tag="pa")
                    nc.tensor.matmul(pa[:], lhs, Mc_sb[:], start=True, stop=True)
                    nc.vector.tensor_copy(hb[:, c.D1:c.D1 + 2 * c.H], pa[:])
                    nc.sync.dma_start(h_d[r * 128:(r + 1) * 128, :], hb[:])

            psA = contextlib.ExitStack()
            psA_pool = psA.enter_context(
                tc.tile_pool(name="psA", bufs=1, space="PSUM"))

            # ============ Phase 1: GAT edge softmax + scatter -> y ============
            with tc.tile_pool(name="p1", bufs=2) as p1, \
                 tc.tile_pool(name="p1h", bufs=2) as p1h, \
                 tc.tile_pool(name="p1o", bufs=2) as p1o, \
                 tc.tile_pool(name="psD", bufs=1, space="PSUM") as psD, \
                 tc.tile_pool(name="psE", bufs=2, space="PSUM") as psE:
                for t in range(c.NT if STAGE >= 2 else 0):
                    cs = t * c.TCT * 8
                    ce = (t + 1) * c.TCT * 8
                    si = p1.tile([128, c.TCT * 8], I16, tag="si")
                    nc.sync.dma_start(si[:], sidx[:, cs:ce])
                    ob = p1o.tile([128, c.TCT, 128], F8, tag="ob")
                    nc.sync.dma_start(ob[:], ohb2[:, t * c.TCT:(t + 1) * c.TCT, :])
                    obw = p1o.tile([128, c.TCT, 128], BF16, tag="obw")
                    nc.sync.dma_start(obw[:], ohb1[:, t * c.TCT:(t + 1) * c.TCT, :])
                    oT = p1o.tile([128, c.TCT, 128], BF16, tag="oT")
                    nc.sync.dma_start(oT[:], ohT[:, t * c.TCT:(t + 1) * c.TCT, :])
                    dn = p1.tile([128, 8], I16, tag="dn")
                    nc.sync.dma_start(dn[:], dnid[:, t * 8:(t + 1) * 8])
                    adt = p1.tile([128, 1, 128], BF16, tag="adt")
                    nc.gpsimd.dma_gather(adt[:], h_d[:, c.DP - 128:c.DP],
                                         dn[:], 128, 128, 128, elem_step=c.DP)

                    px = psA_pool.tile([128, c.DP], F32, tag="px")
                    pd = psD.tile([128, c.H], F32, tag="pd")
                    exf = p1.tile([128, c.TCT, c.H], BF16, tag="exf")
                    exf2 = p1.tile([128, c.TCT, c.H], BF16, tag="exf2")
                    ex8 = p1.tile([128, c.TCT, c.H], F8, tag="ex8")
                    for g in range(c.NGRP):
                        c0 = g * c.GRP
                        c1 = min(c0 + c.GRP, c.TCT)
                        nch = c1 - c0
                        hg = p1h.tile([128, c.GRP, c.DP], BF16, tag="hg")
                        nc.gpsimd.dma_gather(hg[:, 0:nch, :], h_d[:],
                                             si[:, c0 * 8:c1 * 8],
                                             nch * 128, nch * 128, c.DP)
                        peg = psE.tile([128, c.GRP, c.H], F32, tag="peg")
                        for ch in range(c0, c1):
                            nc.tensor.matmul(
                                peg[:, ch - c0, :], oT[:, ch, :],
                                adt[:, 0, 128 - c.H:128],
                                start=True, stop=True)
                        ev = exf[:, c0:c1, :]
                        nc.vector.tensor_add(ev, peg[:, 0:nch, :],
                                             hg[:, 0:nch, c.D1:c.D1 + c.H])
                        ev2 = exf2[:, c0:c1, :]
                        nc.vector.tensor_tensor(
                            ev2, ev,
                            c02[:, :, None].broadcast_to([128, nch, c.H]),
                            Alu.mult)
                        nc.vector.tensor_tensor(ev, ev, ev2, Alu.max)
                        nc.scalar.activation(ev, ev, Act.Exp)
                        nc.scalar.copy(ex8[:, c0:c1, :], ev)
                        mv = hg[:, 0:nch, 0:c.D1].rearrange(
                            "p t (h w) -> p t h w", h=c.H)
                        ebl = exf[:, c0:c1, 0:c.HD, None].broadcast_to(
                            [128, nch, c.HD, c.C])
                        ebp = exf[:, c0:c1, c.HD:c.H, None].broadcast_to(
                            [128, nch, c.H - c.HD, c.C])
                        nc.vector.tensor_mul(mv[:, :, 0:c.HD, :],
                                             mv[:, :, 0:c.HD, :], ebl)
                        if c.HD < c.H:
                            nc.gpsimd.tensor_mul(mv[:, :, c.HD:c.H, :],
                                                 mv[:, :, c.HD:c.H, :], ebp)
                        for ch2 in range(c0, c1, 2):
                            first = (ch2 == 0)
                            last = (ch2 == c.TCT - 2)
                            nc.tensor.matmul(pd[:], ob[:, ch2:ch2 + 2, :],
                                             ex8[:, ch2:ch2 + 2, :],
                                             start=first, stop=last,
                                             perf_mode=DR)
                        for ch in range(c0, c1):
                            for j0 in range(0, c.DP, 512):
                                nc.tensor.matmul(
                                    px[:, j0:j0 + 512], obw[:, ch, :],
                                    hg[:, ch - c0, j0:j0 + 512],
                                    start=(ch == 0), stop=(ch == c.TCT - 1))
                    rdn = p1.tile([128, c.H], F32, tag="rdn")
                    nc.vector.reciprocal(rdn[:], pd[:])
                    sc = p1.tile([128, c.H], F32, tag="sc")
                    nc.vector.tensor_mul(
                        sc[:], rdn[:],
                        scl_sb[:, t:t + 1].broadcast_to([128, c.H]))
                    yt = p1.tile([128, c.DP], F8, tag="yt")
                    nc.vector.memset(yt[:, c.D1:], 0.0)
                    pxv = px[:, 0:c.D1].rearrange("p (h w) -> p h w", h=c.H)
                    ytv = yt[:, 0:c.D1].rearrange("p (h w) -> p h w", h=c.H)
                    scb = sc[:, :, None].broadcast_to([128, c.H, c.C])
                    if not c.B1NZ:
                        nc.vector.scalar_tensor_tensor(ytv, pxv, 0.0, scb,
                                                       Alu.max, Alu.mult)
                    else:
                        x1f = p1.tile([128, c.D1], F32, tag="x1f")
                        x1v = x1f[:].rearrange("p (h w) -> p h w", h=c.H)
                        rb = rdn[:, :, None].broadcast_to([128, c.H, c.C])
                        nc.vector.tensor_mul(x1v, pxv, rb)
                        nc.vector.tensor_add(x1f[:], x1f[:],
                                             bgat_sb[:, 0:c.D1])
                        nc.vector.tensor_scalar_max(x1f[:], x1f[:], 0.0)
                        dvb = scl_sb[:, t:t + 1].broadcast_to([128, c.D1])
                        nc.vector.tensor_tensor(yt[:, 0:c.D1], x1f[:], dvb,
                                                Alu.mult)
                    nc.sync.dma_start(y_d[t * 128:(t + 1) * 128, :], yt[:])
                    # chunked AllGather as soon as a chunk's tiles are done
                    if STAGE >= 3 and (t + 1) % c.TPC == 0:
                        k = (t + 1) // c.TPC - 1
                        r0 = k * c.TPC * 128
                        r1 = (k + 1) * c.TPC * 128
                        nc.gpsimd.collective_compute(
                            "AllGather", Alu.bypass,
                            ins=[y_d[r0:r1, :]],
                            outs=[yf_d[r0 * c.NCORES:r1 * c.NCORES, :]],
                            replica_groups=rg)

            if DEBUG:
                nc.sync.dma_start(dbg_h[:], h_d[:])
                nc.sync.dma_start(dbg_y[:], yf_d[:].bitcast(mybir.dt.uint8))

            # ============ Phase 2: GCN scatter + dense + pooling ============
            with tc.tile_pool(name="p2", bufs=2) as p2, \
                 tc.tile_pool(name="p2h", bufs=2) as p2h, \
                 tc.tile_pool(name="p2o", bufs=2) as p2o, \
                 tc.tile_pool(name="gd", bufs=1) as gd, \
                 tc.tile_pool(name="gw", bufs=2) as gw, \
                 tc.tile_pool(name="psW", bufs=2, space="PSUM") as psW, \
                 tc.tile_pool(name="psP", bufs=1, space="PSUM") as psP:
                gacc = pp.tile([128, c.KS * c.G], F32)
                nc.vector.memset(gacc[:], 0.0)

                for hf in range(c.NHALF if STAGE >= 4 else 0):
                    for t in range(hf * c.HT, (hf + 1) * c.HT):
                        cs = t * c.TCT * 8
                        ce = (t + 1) * c.TCT * 8
                        xi = p2.tile([128, c.TCT * 8], I16, tag="xi")
                        nc.sync.dma_start(xi[:], yidx[:, cs:ce])
                        ob2 = p2o.tile([128, c.TCT, 128], F8, tag="ob2")
                        nc.sync.dma_start(ob2[:],
                                          ohb2[:, t * c.TCT:(t + 1) * c.TCT, :])
                        px2 = psA_pool.tile([128, c.DP], F32, tag="px")
                        for g in range(c.NGRP):
                            c0 = g * c.GRP
                            c1 = min(c0 + c.GRP, c.TCT)
                            nch = c1 - c0
                            yg = p2h.tile([128, c.GRP, c.DP], F8, tag="hg")
                            nc.gpsimd.dma_gather(yg[:, 0:nch, :], yf_d[:],
                                                 xi[:, c0 * 8:c1 * 8],
                                                 nch * 128, nch * 128, c.DP)
                            for ch2 in range(c0, c1, 2):
                                first = (ch2 == 0)
                                last = (ch2 == c.TCT - 2)
                                for j0 in range(0, c.DP, 512):
                                    nc.tensor.matmul(
                                        px2[:, j0:j0 + 512],
                                        ob2[:, ch2:ch2 + 2, :],
                                        yg[:, ch2 - c0:ch2 - c0 + 2,
                                           j0:j0 + 512],
                                        start=first, stop=last, perf_mode=DR)
                        agt = p2.tile([128, c.DP], BF16, tag="agt")
                        nc.scalar.copy(agt[:], px2[:])
                        nc.sync.dma_start(aggb_d[t * 128:(t + 1) * 128, :],
                                          agt[:])
                    if STAGE < 5:
                        continue
                    # dense for this half: transpose agg, matmul, relu*dinv
                    hr0 = hf * c.HT * 128
                    hr1 = (hf + 1) * c.HT * 128
                    aggT = gd.tile([128, c.KS, c.HT * 128], BF16, tag="aT")
                    for k in range(c.KS):
                        nc.sync.dma_start(aggT[:, k, :],
                                          aggb_d[hr0:hr1, k * 128:(k + 1) * 128],
                                          transpose=True)
                    for m in range(hf * c.HT, (hf + 1) * c.HT):
                        mo = (m - hf * c.HT) * 128
                        xt2 = gw.tile([128, c.DP], BF16, tag="xt2")
                        for j0 in range(0, c.DP, 512):
                            pw = psW.tile([128, 512], F32, tag="pw")
                            for k in range(c.KS):
                                nc.tensor.matmul(
                                    pw[:],
                                    aggT[:, k, mo:mo + 128],
                                    wgcn_sb[:, k, j0:j0 + 512],
                                    start=(k == 0), stop=(k == c.KS - 1))
                            if not c.B2NZ:
                                nc.scalar.activation(
                                    xt2[:, j0:j0 + 512], pw[:],
                                    Act.Relu, scale=scl_sb[:, m:m + 1])
                            else:
                                xf = gw.tile([128, 512], F32, tag="xf")
                                dvb = scl_sb[:, m:m + 1].broadcast_to(
                                    [128, 512])
                                nc.vector.tensor_tensor(
                                    xf[:], pw[:], dvb, Alu.mult)
                                nc.vector.tensor_add(
                                    xf[:], xf[:], bgcn_sb[:, j0:j0 + 512])
                                nc.vector.tensor_scalar_max(
                                    xt2[:, j0:j0 + 512], xf[:], 0.0)
                        if DEBUG:
                            nc.sync.dma_start(
                                dbg_x2[m * 128:(m + 1) * 128, :], xt2[:])
                        for fb in range(0, c.KS, 8):
                            fe = min(fb + 8, c.KS)
                            pgt = psP.tile([128, 8, c.G], F32, tag="pg")
                            for fs in range(fb, fe):
                                nc.tensor.matmul(
                                    pgt[:, fs - fb, :],
                                    xt2[:, fs * 128:(fs + 1) * 128],
                                    gon_sb[:, m, :],
                                    start=True, stop=True)
                            nc.vector.tensor_add(
                                gacc[:, fb * c.G:fe * c.G],
                                gacc[:, fb * c.G:fe * c.G],
                                pgt[:, 0:fe - fb, :].rearrange(
                                    "p k g -> p (k g)"))
                nc.gpsimd.dma_start(gs_in_d[:], gacc[:])
                if DEBUG:
                    nc.sync.dma_start(dbg_a[:], aggb_d[:])
                    nc.sync.dma_start(dbg_g[:], gs_in_d[:])
            psA.close()

            # ============ AllReduce pooled sums + FC ============
            if STAGE >= 6:
                nc.gpsimd.collective_compute(
                    "AllReduce", Alu.add, ins=[gs_in_d[:]], outs=[gs_out_d[:]],
                    replica_groups=rg)
            with tc.tile_pool(name="fc", bufs=1) as fc, \
                 tc.tile_pool(name="psS", bufs=1, space="PSUM") as psS:
              if STAGE < 6:
                dz = fc.tile([c.G, c.OUT], F32)
                nc.vector.memset(dz[:], 0.0)
                nc.sync.dma_start(out[:], dz[:])
              else:
                gsar = fc.tile([128, c.KS, c.G], F32)
                nc.sync.dma_start(gsar[:],
                                  gs_out_d[:].rearrange("p (k g) -> p k g",
                                                        k=c.KS))
                iv_sb = fc.tile([128, c.G], F32)
                nc.sync.dma_start(iv_sb[:], invcnt[:])
                gm = fc.tile([128, c.KS, c.G], F32)
                nc.vector.tensor_mul(
                    gm[:], gsar[:],
                    iv_sb[:, None, :].broadcast_to([128, c.KS, c.G]))
                wf_sb = fc.tile([128, c.FCK, c.OUT], F32)
                nc.sync.dma_start(
                    wf_sb[:], Wfc[:].rearrange("(k p) o -> p k o", p=128))
                pf = psS.tile([c.G, c.OUT], F32, tag="sm")
                for k in range(c.FCK):
                    lhs = gm[:, k, :] if k < c.KS else gsar[:, k - c.KS, :]
                    nc.tensor.matmul(pf[:], lhs, wf_sb[:, k, :],
                                     start=(k == 0), stop=(k == c.FCK - 1))
                bf_sb = fc.tile([c.G, c.OUT], F32)
                nc.sync.dma_start(bf_sb[:], bfc[:])
                ot = fc.tile([c.G, c.OUT], F32)
                nc.vector.tensor_add(ot[:], pf[:], bf_sb[:])
                nc.vector.tensor_scalar_max(ot[:], ot[:], 0.0)
                nc.sync.dma_start(out[:], ot[:])

    nc.compile()
    return nc


# ================= host-side preprocessing =================

def _wrap_idx(a):
    """[L] int -> [128, L//16] int16 wrapped (i -> [i%16, i//16]) + 8x repl."""
    w = a.reshape(-1, 16).T.astype(np.int16)
    return np.tile(w, (8, 1)).copy()


def preprocess(x, edge_index, batch, num_graphs, W_gat, att_src, att_dst,
               b_gat, W_gcn, b_gcn, W_fc, b_fc, cfg=None, ncores=8):
    N, C = x.shape
    E = edge_index.shape[1]
    H = att_src.shape[0]
    G = int(num_graphs)
    OUT = W_fc.shape[1]

    src = np.concatenate([np.asarray(edge_index[0]), np.arange(N)]).astype(np.int64)
    dst = np.concatenate([np.asarray(edge_index[1]), np.arange(N)]).astype(np.int64)
    deg = np.bincount(dst, minlength=N).astype(np.float32)
    dinv = np.where(deg > 0, 1.0 / np.sqrt(deg), 0.0).astype(np.float32)

    NC_ = ncores
    NPC = _ru(N, NC_) // NC_
    NT = _ru(NPC, 128) // 128

    order = np.argsort(dst, kind='stable')
    s_s, s_d = src[order], dst[order]

    # per (core,tile) edge lists
    tiles = [[None] * NT for _ in range(NC_)]
    for core in range(NC_):
        for t in range(NT):
            lo = np.searchsorted(s_d, core * NPC + t * 128)
            hi = np.searchsorted(s_d, min(core * NPC + (t + 1) * 128,
                                          (core + 1) * NPC))
            tiles[core][t] = (s_s[lo:hi], s_d[lo:hi])

    TCT = max(max(_ru(len(tt[0]), 128) // 128 for tt in row) for row in tiles)
    TCT = max(_ru(TCT, 2), 2)
    if cfg is None:
        cfg = Cfg(N, E, H, C, G, OUT, TCT, NCORES=NC_)
        cfg.B1NZ = bool(np.any(np.asarray(b_gat) != 0))
        cfg.B2NZ = bool(np.any(np.asarray(b_gcn) != 0))
    assert cfg.TCT == TCT

    c = cfg
    # replicated tensors
    xT = np.zeros((C, c.NPAD), BF)
    xT[:, :N] = np.asarray(x).T.astype(BF)
    Wgf = np.asarray(W_gat).astype(np.float32)
    Wg = Wgf.astype(BF)
    Wg3 = Wgf.reshape(C, H, C)
    Mcat = np.zeros((C, 2 * H), BF)
    Mcat[:, 0:H] = np.einsum('khc,hc->kh', Wg3, np.asarray(att_src)).astype(BF)
    Mcat[:, H:2 * H] = np.einsum('khc,hc->kh', Wg3, np.asarray(att_dst)).astype(BF)
    bgat = np.zeros((128, c.DP), np.float32)
    bgat[:, :c.D1] = np.asarray(b_gat)[None, :]
    bgcn = np.zeros((128, c.DP), np.float32)
    bgcn[:, :c.D1] = np.asarray(b_gcn)[None, :]
    Wgcn = np.zeros((c.DP, c.DP), BF)
    Wgcn[:c.D1, :c.D1] = np.asarray(W_gcn).astype(BF)
    Wfc = np.zeros((2 * c.DP, OUT), np.float32)
    Wfc[0:c.D1] = np.asarray(W_fc)[0:c.D1]
    Wfc[c.DP:c.DP + c.D1] = np.asarray(W_fc)[c.D1:2 * c.D1]
    bfc = np.tile(np.asarray(b_fc).astype(np.float32)[None, :], (G, 1))
    cnt = np.bincount(np.asarray(batch), minlength=G).astype(np.float32)
    invcnt = np.tile((1.0 / np.maximum(cnt, 1.0))[None, :], (128, 1))

    batch_np = np.asarray(batch)
    shared = dict(xT=xT, Wg=Wg, Mcat=Mcat, Wgcn=Wgcn, Wfc=Wfc, bfc=bfc,
                  invcnt=invcnt, bgat=bgat, bgcn=bgcn)

    # y row index in the chunk-wise AllGathered layout, per source node id
    def yrow_of(j, core_of):
        local = j - core_of * NPC
        t = local // 128
        r = local % 128
        k = t // c.TPC
        return (k * c.NCORES * c.TPC * 128 + core_of * c.TPC * 128
                + (t - k * c.TPC) * 128 + r)

    in_maps = []
    for core in range(NC_):
        L = c.TC * 128
        sp = np.zeros(L, np.int64)
        dl = np.zeros(L, np.int64)
        valid = np.zeros(L, bool)
        for t in range(NT):
            ts, td = tiles[core][t]
            o = t * c.TCT * 128
            k = len(ts)
            sp[o:o + k] = ts
            dl[o:o + k] = td - (core * NPC + t * 128)
            valid[o:o + k] = True
        cs = sp // NPC
        yr = np.array([yrow_of(j, cj) for j, cj in zip(sp, cs)], np.int64)
        oh = np.zeros((c.TC, 128, 128), np.float32)
        ee = np.arange(L)
        oh[ee // 128, ee % 128, dl] = valid.astype(np.float32)
        # this core's dst-node ids per (tile, slot), clamped to valid rows
        dnids = np.zeros((NT, 128), np.int64)
        for t in range(NT):
            gids = core * NPC + t * 128 + np.arange(128)
            dnids[t] = np.minimum(gids, N - 1)
        sclm = np.zeros((128, NT), np.float32)
        for t in range(NT):
            gids = core * NPC + t * 128 + np.arange(128)
            ok = gids < min((core + 1) * NPC, N)
            sclm[ok, t] = dinv[gids[ok]]
        gonm = np.zeros((128, NT, G), BF)
        for t in range(NT):
            gids = core * NPC + t * 128 + np.arange(128)
            ok = gids < min((core + 1) * NPC, N)
            gonm[ok, t, batch_np[gids[ok]]] = 1.0
        m = dict(shared)
        m.update(
            sidx=_wrap_idx(sp), yidx=_wrap_idx(yr),
            dnid=_wrap_idx(dnids.reshape(-1)),
            ohb1=oh.transpose(1, 0, 2).astype(BF),
            ohb2=oh.transpose(1, 0, 2).astype(NPF8),
            ohT=oh.transpose(2, 0, 1).astype(BF),
            scl=sclm, gon=gonm)
        in_maps.append(m)
    return cfg, in_maps


_CACHE = {}


def run(inputs, trace=False):
    key = tuple(sorted((k, tuple(np.shape(v))) for k, v in inputs.items()))
    cfg, in_maps = preprocess(**inputs,
                              cfg=_CACHE[key][0] if key in _CACHE else None)
    if key not in _CACHE:
        _CACHE[key] = (cfg, build(cfg))
    cfg, nc = _CACHE[key]
    res = run_bass_kernel_spmd(nc, in_maps, core_ids=list(range(cfg.NCORES)),
                               trace=trace)
    return res.results[0]["out"].astype(np.float32), res


def kernel(**inputs):
    out, _ = run(inputs)
    return out


# revision 46
# speedup vs baseline: 1.2602x; 1.0438x over previous
"""GAT+GCN+pool GNN on 8 Trainium2 NeuronCores (Bass/Tile), fp8 edition.

Sharding: nodes/edges partitioned across 8 cores by destination-node range;
segment softmax and scatter-adds are core-local.  Per-edge row gathers use
dma_gather on fp8 rows (h stored as [2496 h | 32 a_src | 32 a_dst] fp8e4),
scatter-adds are DoubleRow fp8 one-hot matmuls (256 edges per pass).

GCN is computed as (A_hat x1) W (associativity) so the only big exchange is
an AllGather of the dinv-prescaled GAT output y = dinv*x1 in fp8 (26MB),
issued in chunks overlapped with phase-1 compute.  The same one-hot tensor
drives both scatter phases.  Dense GCN (bf16) runs per half-graph interleaved
with phase-2 scatter; graph pooling accumulates in PSUM across tiles.

Pipeline (per core, one NEFF):
  A)  h = x @ W_gat (bf16, replicated), a_src/a_dst folded matmul -> fp8 h_d
  1)  per dst-tile: gather fp8 rows per edge -> logits -> exp ->
      exp*h via DVE+GpSimd split -> DoubleRow one-hot scatter -> y (fp8)
  AG) chunked AllGather of y
  2)  per half: gather y rows, DoubleRow one-hot scatter -> agg; DMA-transpose;
      dense agg @ W_gcn (bf16) with fused relu*dinv; pooling matmul in PSUM
  AR) AllReduce pooled sums, gmean, FC, relu -> out [G, OUT]
"""

import sys
import os
import contextlib

if '/opt/trn_rl_repo' not in sys.path:
    sys.path.insert(0, '/opt/trn_rl_repo')

import numpy as np
import ml_dtypes

import concourse.bacc as bacc
import concourse.mybir as mybir
import concourse.tile as tile
from concourse.bass_utils import run_bass_kernel_spmd

F32 = mybir.dt.float32
BF16 = mybir.dt.bfloat16
F8 = mybir.dt.float8e4
I16 = mybir.dt.int16
BF = ml_dtypes.bfloat16
NPF8 = ml_dtypes.float8_e4m3
Alu = mybir.AluOpType
Act = mybir.ActivationFunctionType
DR = mybir.MatmulPerfMode.DoubleRow


def _ru(x, m):
    return (x + m - 1) // m * m


class Cfg:
    def __init__(self, N, E, H, C, G, OUT, TCT, NCORES=8, GRP=6, HD=22, AGC=1):
        self.N, self.E, self.H, self.C, self.G, self.OUT = N, E, H, C, G, OUT
        self.NCORES = NCORES
        self.D1 = H * C                              # 2496
        self.DP = _ru(self.D1 + 2 * H, 128)          # 2560 fp8 row bytes
        assert self.DP % 256 == 0
        self.NPC = _ru(N, NCORES) // NCORES          # nodes per core
        self.NT = _ru(self.NPC, 128) // 128          # dst tiles per core
        self.XWROWS = self.NT * 128
        self.XWFULL = NCORES * self.XWROWS
        self.ROWS_A = _ru(N, 128) // 128             # stage-A node tiles
        self.NPAD = self.ROWS_A * 128
        self.KS = self.DP // 128                     # dense k slabs
        self.FCK = 2 * self.KS
        assert TCT % 2 == 0
        self.TCT = TCT                               # chunks per dst tile
        self.TC = self.NT * TCT
        self.GRP = GRP                               # chunks per gather group
        assert GRP % 2 == 0
        self.NGRP = (TCT + GRP - 1) // GRP
        self.HD = HD                                 # heads multiplied on DVE
        self.AGC = AGC                               # allgather chunks
        assert self.NT % AGC == 0
        self.TPC = self.NT // AGC                    # tiles per AG chunk
        self.NHALF = 2                               # dense half-phases
        assert self.NT % self.NHALF == 0
        self.HT = self.NT // self.NHALF              # tiles per half
        self.B1NZ = False                            # b_gat nonzero
        self.B2NZ = False                            # b_gcn nonzero


def build(cfg):
    STAGE = int(os.environ.get("GNN_STAGE", "9"))
    DEBUG = int(os.environ.get("GNN_DEBUG", "0"))
    MV = int(os.environ.get("GNN_MV", "0"))
    CD = int(os.environ.get("GNN_CD", "4"))
    hd_env = os.environ.get("GNN_HD")
    if hd_env is not None:
        cfg.HD = int(hd_env)
    c = cfg
    nc = bacc.Bacc(None, target_bir_lowering=False)

    # ---- external inputs (replicated unless noted per-core) ----
    xT = nc.dram_tensor("xT", [c.C, c.NPAD], BF16, kind="ExternalInput")
    Wg = nc.dram_tensor("Wg", [c.C, c.D1], BF16, kind="ExternalInput")
    Mcat = nc.dram_tensor("Mcat", [c.C, 2 * c.H], BF16, kind="ExternalInput")
    Wgcn = nc.dram_tensor("Wgcn", [c.DP, c.DP], BF16, kind="ExternalInput")
    Wfc = nc.dram_tensor("Wfc", [2 * c.DP, c.OUT], F32, kind="ExternalInput")
    bfc = nc.dram_tensor("bfc", [c.G, c.OUT], F32, kind="ExternalInput")
    bgat = nc.dram_tensor("bgat", [128, c.DP], F32, kind="ExternalInput")
    bgcn = nc.dram_tensor("bgcn", [128, c.DP], F32, kind="ExternalInput")
    invcnt = nc.dram_tensor("invcnt", [128, c.G], F32, kind="ExternalInput")
    # per-core:
    sidx = nc.dram_tensor("sidx", [128, c.TC * 8], I16, kind="ExternalInput")
    yidx = nc.dram_tensor("yidx", [128, c.TC * 8], I16, kind="ExternalInput")
    dnid = nc.dram_tensor("dnid", [128, c.NT * 8], I16, kind="ExternalInput")
    ohb1 = nc.dram_tensor("ohb1", [128, c.TC, 128], BF16, kind="ExternalInput")
    ohb2 = nc.dram_tensor("ohb2", [128, c.TC, 128], F8, kind="ExternalInput")
    ohT = nc.dram_tensor("ohT", [128, c.TC, 128], BF16, kind="ExternalInput")
    scl = nc.dram_tensor("scl", [128, c.NT], F32, kind="ExternalInput")
    gon = nc.dram_tensor("gon", [128, c.NT, c.G], BF16, kind="ExternalInput")
    out = nc.dram_tensor("out", [c.G, c.OUT], F32, kind="ExternalOutput")
    if DEBUG:
        dbg_h = nc.dram_tensor("dbg_h", [c.NPAD, c.DP], BF16,
                               kind="ExternalOutput")
        dbg_y = nc.dram_tensor("dbg_y", [c.XWFULL, c.DP], mybir.dt.uint8,
                               kind="ExternalOutput")
        dbg_a = nc.dram_tensor("dbg_a", [c.XWROWS, c.DP], BF16,
                               kind="ExternalOutput")
        dbg_g = nc.dram_tensor("dbg_g", [128, c.KS * c.G], F32,
                               kind="ExternalOutput")
        dbg_x2 = nc.dram_tensor("dbg_x2", [c.XWROWS, c.DP], BF16,
                                kind="ExternalOutput")

    rg = [list(range(c.NCORES))]

    with tile.TileContext(nc) as tc:
        with (
            tc.tile_pool(name="dram", bufs=1, space="DRAM") as dram,
            tc.tile_pool(name="persist", bufs=1) as pp,
        ):
            h_d = dram.tile([c.NPAD, c.DP], BF16)
            y_d = dram.tile([c.XWROWS, c.DP], F8)
            yf_d = dram.tile([c.XWFULL, c.DP], F8, addr_space="Shared")
            aggb_d = dram.tile([c.XWROWS, c.DP], BF16)
            gs_in_d = dram.tile([128, c.KS * c.G], F32)
            gs_out_d = dram.tile([128, c.KS * c.G], F32, addr_space="Shared")

            # persistent smalls + resident GCN weights
            scl_sb = pp.tile([128, c.NT], F32)
            nc.sync.dma_start(scl_sb[:], scl[:])
            c02 = pp.tile([128, 1], BF16)
            nc.vector.memset(c02[:], 0.2)
            gon_sb = pp.tile([128, c.NT, c.G], BF16)
            nc.sync.dma_start(gon_sb[:], gon[:])
            wgcn_sb = pp.tile([128, c.KS, c.DP], BF16)
            for k in range(c.KS):
                nc.sync.dma_start(wgcn_sb[:, k, :],
                                  Wgcn[k * 128:(k + 1) * 128, :])
            if c.B1NZ:
                bgat_sb = pp.tile([128, c.DP], F32)
                nc.sync.dma_start(bgat_sb[:], bgat[:])
            if c.B2NZ:
                bgcn_sb = pp.tile([128, c.DP], F32)
                nc.sync.dma_start(bgcn_sb[:], bgcn[:])

            # ============ Stage A: h = x@Wg -> fp8 h_d with a-tail ============
            with tc.tile_pool(name="stageA", bufs=2) as sa, \
                 tc.tile_pool(name="stageAc", bufs=1) as sac, \
                 tc.tile_pool(name="psH", bufs=5, space="PSUM") as psH, \
                 tc.tile_pool(name="psHa", bufs=2, space="PSUM") as psHa:
                xT_sb = sac.tile([c.C, c.NPAD], BF16)
                nc.sync.dma_start(xT_sb[:], xT[:])
                Wg_sb = sac.tile([c.C, c.D1], BF16)
                nc.sync.dma_start(Wg_sb[:], Wg[:])
                Mc_sb = sac.tile([c.C, 2 * c.H], BF16)
                nc.sync.dma_start(Mc_sb[:], Mcat[:])
                for r in range(c.ROWS_A if STAGE >= 1 else 0):
                    lhs = xT_sb[:, r * 128:(r + 1) * 128]
                    hb = sa.tile([128, c.DP], BF16, tag="hb")
                    for i, j0 in enumerate(range(0, c.D1, 512)):
                        j1 = min(j0 + 512, c.D1)
                        ph = psH.tile([128, 512], F32, tag="ph")
                        nc.tensor.matmul(ph[:, 0:j1 - j0], lhs, Wg_sb[:, j0:j1],
                                         start=True, stop=True)
                        if i % 2 == 0:
                            nc.scalar.copy(hb[:, j0:j1], ph[:, 0:j1 - j0])
                        else:
                            nc.vector.tensor_copy(hb[:, j0:j1], ph[:, 0:j1 - j0])
                    pa = psHa.tile([128, 2 * c.H], F32, tag="pa")
                    nc.tensor.matmul(pa[:], lhs, Mc_sb[:], start=True, stop=True)
                    nc.vector.tensor_copy(hb[:, c.D1:c.D1 + 2 * c.H], pa[:])
                    nc.sync.dma_start(h_d[r * 128:(r + 1) * 128, :], hb[:])

            psA = contextlib.ExitStack()
            psA_pool = psA.enter_context(
                tc.tile_pool(name="psA", bufs=1, space="PSUM"))

            # ============ Phase 1: GAT edge softmax + scatter -> y ============
            with tc.tile_pool(name="p1", bufs=2) as p1, \
                 tc.tile_pool(name="p1h", bufs=2) as p1h, \
                 tc.tile_pool(name="p1o", bufs=2) as p1o, \
                 tc.tile_pool(name="psD", bufs=1, space="PSUM") as psD, \
                 tc.tile_pool(name="psE", bufs=2, space="PSUM") as psE:
                for t in range(c.NT if STAGE >= 2 else 0):
                    cs = t * c.TCT * 8
                    ce = (t + 1) * c.TCT * 8
                    si = p1.tile([128, c.TCT * 8], I16, tag="si")
                    nc.sync.dma_start(si[:], sidx[:, cs:ce])
                    ob = p1o.tile([128, c.TCT, 128], F8, tag="ob")
                    nc.sync.dma_start(ob[:], ohb2[:, t * c.TCT:(t + 1) * c.TCT, :])
                    obw = p1o.tile([128, c.TCT, 128], BF16, tag="obw")
                    nc.sync.dma_start(obw[:], ohb1[:, t * c.TCT:(t + 1) * c.TCT, :])
                    oT = p1o.tile([128, c.TCT, 128], BF16, tag="oT")
                    nc.sync.dma_start(oT[:], ohT[:, t * c.TCT:(t + 1) * c.TCT, :])
                    dn = p1.tile([128, 8], I16, tag="dn")
                    nc.sync.dma_start(dn[:], dnid[:, t * 8:(t + 1) * 8])
                    adt = p1.tile([128, 1, 128], BF16, tag="adt")
                    nc.gpsimd.dma_gather(adt[:], h_d[:, c.DP - 128:c.DP],
                                         dn[:], 128, 128, 128, elem_step=c.DP)

                    px = psA_pool.tile([128, c.DP], F32, tag="px")
                    pd = psD.tile([128, c.H], F32, tag="pd")
                    exf = p1.tile([128, c.TCT, c.H], BF16, tag="exf")
                    exf2 = p1.tile([128, c.TCT, c.H], BF16, tag="exf2")
                    ex8 = p1.tile([128, c.TCT, c.H], F8, tag="ex8")
                    for g in range(c.NGRP):
                        c0 = g * c.GRP
                        c1 = min(c0 + c.GRP, c.TCT)
                        nch = c1 - c0
                        hg = p1h.tile([128, c.GRP, c.DP], BF16, tag="hg")
                        nc.gpsimd.dma_gather(hg[:, 0:nch, :], h_d[:],
                                             si[:, c0 * 8:c1 * 8],
                                             nch * 128, nch * 128, c.DP)
                        peg = psE.tile([128, c.GRP, c.H], F32, tag="peg")
                        for ch in range(c0, c1):
                            nc.tensor.matmul(
                                peg[:, ch - c0, :], oT[:, ch, :],
                                adt[:, 0, 128 - c.H:128],
                                start=True, stop=True)
                        ev = exf[:, c0:c1, :]
                        nc.vector.tensor_add(ev, peg[:, 0:nch, :],
                                             hg[:, 0:nch, c.D1:c.D1 + c.H])
                        ev2 = exf2[:, c0:c1, :]
                        nc.vector.tensor_tensor(
                            ev2, ev,
                            c02[:, :, None].broadcast_to([128, nch, c.H]),
                            Alu.mult)
                        nc.vector.tensor_tensor(ev, ev, ev2, Alu.max)
                        nc.scalar.activation(ev, ev, Act.Exp)
                        nc.scalar.copy(ex8[:, c0:c1, :], ev)
                        mv = hg[:, 0:nch, 0:c.D1].rearrange(
                            "p t (h w) -> p t h w", h=c.H)
                        ebl = exf[:, c0:c1, 0:c.HD, None].broadcast_to(
                            [128, nch, c.HD, c.C])
                        ebp = exf[:, c0:c1, c.HD:c.H, None].broadcast_to(
                            [128, nch, c.H - c.HD, c.C])
                        nc.vector.tensor_mul(mv[:, :, 0:c.HD, :],
                                             mv[:, :, 0:c.HD, :], ebl)
                        if c.HD < c.H:
                            nc.gpsimd.tensor_mul(mv[:, :, c.HD:c.H, :],
                                                 mv[:, :, c.HD:c.H, :], ebp)
                        for ch2 in range(c0, c1, 2):
                            first = (ch2 == 0)
                            last = (ch2 == c.TCT - 2)
                            nc.tensor.matmul(pd[:], ob[:, ch2:ch2 + 2, :],
                                             ex8[:, ch2:ch2 + 2, :],
                                             start=first, stop=last,
                                             perf_mode=DR)
                        for ch in range(c0, c1):
                            for j0 in range(0, c.DP, 512):
                                nc.tensor.matmul(
                                    px[:, j0:j0 + 512], obw[:, ch, :],
                                    hg[:, ch - c0, j0:j0 + 512],
                                    start=(ch == 0), stop=(ch == c.TCT - 1))
                    rdn = p1.tile([128, c.H], F32, tag="rdn")
                    nc.vector.reciprocal(rdn[:], pd[:])
                    sc = p1.tile([128, c.H], F32, tag="sc")
                    nc.vector.tensor_mul(
                        sc[:], rdn[:],
                        scl_sb[:, t:t + 1].broadcast_to([128, c.H]))
                    yt = p1.tile([128, c.DP], F8, tag="yt")
                    nc.vector.memset(yt[:, c.D1:], 0.0)
                    pxv = px[:, 0:c.D1].rearrange("p (h w) -> p h w", h=c.H)
                    ytv = yt[:, 0:c.D1].rearrange("p (h w) -> p h w", h=c.H)
                    scb = sc[:, :, None].broadcast_to([128, c.H, c.C])
                    if not c.B1NZ:
                        nc.vector.scalar_tensor_tensor(ytv, pxv, 0.0, scb,
                                                       Alu.max, Alu.mult)
                    else:
                        x1f = p1.tile([128, c.D1], F32, tag="x1f")
                        x1v = x1f[:].rearrange("p (h w) -> p h w", h=c.H)
                        rb = rdn[:, :, None].broadcast_to([128, c.H, c.C])
                        nc.vector.tensor_mul(x1v, pxv, rb)
                        nc.vector.tensor_add(x1f[:], x1f[:],
                                             bgat_sb[:, 0:c.D1])
                        nc.vector.tensor_scalar_max(x1f[:], x1f[:], 0.0)
                        dvb = scl_sb[:, t:t + 1].broadcast_to([128, c.D1])
                        nc.vector.tensor_tensor(yt[:, 0:c.D1], x1f[:], dvb,
                                                Alu.mult)
                    nc.sync.dma_start(y_d[t * 128:(t + 1) * 128, :], yt[:])
                    # chunked AllGather as soon as a chunk's tiles are done
                    if STAGE >= 3 and (t + 1) % c.TPC == 0:
                        k = (t + 1) // c.TPC - 1
                        r0 = k * c.TPC * 128
                        r1 = (k + 1) * c.TPC * 128
                        nc.gpsimd.collective_compute(
                            "AllGather", Alu.bypass,
                            ins=[y_d[r0:r1, :]],
                            outs=[yf_d[r0 * c.NCORES:r1 * c.NCORES, :]],
                            replica_groups=rg)

            if DEBUG:
                nc.sync.dma_start(dbg_h[:], h_d[:])
                nc.sync.dma_start(dbg_y[:], yf_d[:].bitcast(mybir.dt.uint8))

            # ============ Phase 2: GCN scatter + dense + pooling ============
            with tc.tile_pool(name="p2", bufs=2) as p2, \
                 tc.tile_pool(name="p2h", bufs=2) as p2h, \
                 tc.tile_pool(name="p2o", bufs=2) as p2o, \
                 tc.tile_pool(name="gd", bufs=1) as gd, \
                 tc.tile_pool(name="gw", bufs=2) as gw, \
                 tc.tile_pool(name="psW", bufs=2, space="PSUM") as psW, \
                 tc.tile_pool(name="psP", bufs=1, space="PSUM") as psP:
                gacc = pp.tile([128, c.KS * c.G], F32)
                nc.vector.memset(gacc[:], 0.0)

                for hf in range(c.NHALF if STAGE >= 4 else 0):
                    for t in range(hf * c.HT, (hf + 1) * c.HT):
                        cs = t * c.TCT * 8
                        ce = (t + 1) * c.TCT * 8
                        xi = p2.tile([128, c.TCT * 8], I16, tag="xi")
                        nc.sync.dma_start(xi[:], yidx[:, cs:ce])
                        ob2 = p2o.tile([128, c.TCT, 128], F8, tag="ob2")
                        nc.sync.dma_start(ob2[:],
                                          ohb2[:, t * c.TCT:(t + 1) * c.TCT, :])
                        px2 = psA_pool.tile([128, c.DP], F32, tag="px")
                        for g in range(c.NGRP):
                            c0 = g * c.GRP
                            c1 = min(c0 + c.GRP, c.TCT)
                            nch = c1 - c0
                            yg = p2h.tile([128, c.GRP, c.DP], F8, tag="hg")
                            nc.gpsimd.dma_gather(yg[:, 0:nch, :], yf_d[:],
                                                 xi[:, c0 * 8:c1 * 8],
                                                 nch * 128, nch * 128, c.DP)
                            for ch2 in range(c0, c1, 2):
                                first = (ch2 == 0)
                                last = (ch2 == c.TCT - 2)
                                for j0 in range(0, c.DP, 512):
                                    nc.tensor.matmul(
                                        px2[:, j0:j0 + 512],
                                        ob2[:, ch2:ch2 + 2, :],
                                        yg[:, ch2 - c0:ch2 - c0 + 2,
                                           j0:j0 + 512],
                                        start=first, stop=last, perf_mode=DR)
                        agt = p2.tile([128, c.DP], BF16, tag="agt")
                        nc.scalar.copy(agt[:], px2[:])
                        nc.sync.dma_start(aggb_d[t * 128:(t + 1) * 128, :],
                                          agt[:])
                    if STAGE < 5:
                        continue
                    # dense for this half: transpose agg, matmul, relu*dinv
                    hr0 = hf * c.HT * 128
                    hr1 = (hf + 1) * c.HT * 128
                    aggT = gd.tile([128, c.KS, c.HT * 128], BF16, tag="aT")
                    for k in range(c.KS):
                        nc.sync.dma_start(aggT[:, k, :],
                                          aggb_d[hr0:hr1, k * 128:(k + 1) * 128],
                                          transpose=True)
                    for m in range(hf * c.HT, (hf + 1) * c.HT):
                        mo = (m - hf * c.HT) * 128
                        xt2 = gw.tile([128, c.DP], BF16, tag="xt2")
                        for j0 in range(0, c.DP, 512):
                            pw = psW.tile([128, 512], F32, tag="pw")
                            for k in range(c.KS):
                                nc.tensor.matmul(
                                    pw[:],
                                    aggT[:, k, mo:mo + 128],
                                    wgcn_sb[:, k, j0:j0 + 512],
                                    start=(k == 0), stop=(k == c.KS - 1))
                            if not c.B2NZ:
                                nc.scalar.activation(
                                    xt2[:, j0:j0 + 512], pw[:],
                                    Act.Relu, scale=scl_sb[:, m:m + 1])
                            else:
                                xf = gw.tile([128, 512], F32, tag="xf")
                                dvb = scl_sb[:, m:m + 1].broadcast_to(
                                    [128, 512])
                                nc.vector.tensor_tensor(
                                    xf[:], pw[:], dvb, Alu.mult)
                                nc.vector.tensor_add(
                                    xf[:], xf[:], bgcn_sb[:, j0:j0 + 512])
                                nc.vector.tensor_scalar_max(
                                    xt2[:, j0:j0 + 512], xf[:], 0.0)
                        if DEBUG:
                            nc.sync.dma_start(
                                dbg_x2[m * 128:(m + 1) * 128, :], xt2[:])
                        for fb in range(0, c.KS, 8):
                            fe = min(fb + 8, c.KS)
                            pgt = psP.tile([128, 8, c.G], F32, tag="pg")
                            for fs in range(fb, fe):
                                nc.tensor.matmul(
                                    pgt[:, fs - fb, :],
                                    xt2[:, fs * 128:(fs + 1) * 128],
                                    gon_sb[:, m, :],
                                    start=True, stop=True)
                            nc.vector.tensor_add(
                                gacc[:, fb * c.G:fe * c.G],
                                gacc[:, fb * c.G:fe * c.G],
                                pgt[:, 0:fe - fb, :].rearrange(
                                    "p k g -> p (k g)"))
                nc.gpsimd.dma_start(gs_in_d[:], gacc[:])
                if DEBUG:
                    nc.sync.dma_start(dbg_a[:], aggb_d[:])
                    nc.sync.dma_start(dbg_g[:], gs_in_d[:])
            psA.close()

            # ============ AllReduce pooled sums + FC ============
            if STAGE >= 6:
                nc.gpsimd.collective_compute(
                    "AllReduce", Alu.add, ins=[gs_in_d[:]], outs=[gs_out_d[:]],
                    replica_groups=rg)
            with tc.tile_pool(name="fc", bufs=1) as fc, \
                 tc.tile_pool(name="psS", bufs=1, space="PSUM") as psS:
              if STAGE < 6:
                dz = fc.tile([c.G, c.OUT], F32)
                nc.vector.memset(dz[:], 0.0)
                nc.sync.dma_start(out[:], dz[:])
              else:
                gsar = fc.tile([128, c.KS, c.G], F32)
                nc.sync.dma_start(gsar[:],
                                  gs_out_d[:].rearrange("p (k g) -> p k g",
                                                        k=c.KS))
                iv_sb = fc.tile([128, c.G], F32)
                nc.sync.dma_start(iv_sb[:], invcnt[:])
                gm = fc.tile([128, c.KS, c.G], F32)
                nc.vector.tensor_mul(
                    gm[:], gsar[:],
                    iv_sb[:, None, :].broadcast_to([128, c.KS, c.G]))
                wf_sb = fc.tile([128, c.FCK, c.OUT], F32)
                nc.sync.dma_start(
                    wf_sb[:], Wfc[:].rearrange("(k p) o -> p k o", p=128))
                pf = psS.tile([c.G, c.OUT], F32, tag="sm")
                for k in range(c.FCK):
                    lhs = gm[:, k, :] if k < c.KS else gsar[:, k - c.KS, :]
                    nc.tensor.matmul(pf[:], lhs, wf_sb[:, k, :],
                                     start=(k == 0), stop=(k == c.FCK - 1))
                bf_sb = fc.tile([c.G, c.OUT], F32)
                nc.sync.dma_start(bf_sb[:], bfc[:])
                ot = fc.tile([c.G, c.OUT], F32)
                nc.vector.tensor_add(ot[:], pf[:], bf_sb[:])
                nc.vector.tensor_scalar_max(ot[:], ot[:], 0.0)
                nc.sync.dma_start(out[:], ot[:])

    nc.compile()
    return nc


# ================= host-side preprocessing =================

def _wrap_idx(a):
    """[L] int -> [128, L//16] int16 wrapped (i -> [i%16, i//16]) + 8x repl."""
    w = a.reshape(-1, 16).T.astype(np.int16)
    return np.tile(w, (8, 1)).copy()


def preprocess(x, edge_index, batch, num_graphs, W_gat, att_src, att_dst,
               b_gat, W_gcn, b_gcn, W_fc, b_fc, cfg=None, ncores=8):
    N, C = x.shape
    E = edge_index.shape[1]
    H = att_src.shape[0]
    G = int(num_graphs)
    OUT = W_fc.shape[1]

    src = np.concatenate([np.asarray(edge_index[0]), np.arange(N)]).astype(np.int64)
    dst = np.concatenate([np.asarray(edge_index[1]), np.arange(N)]).astype(np.int64)
    deg = np.bincount(dst, minlength=N).astype(np.float32)
    dinv = np.where(deg > 0, 1.0 / np.sqrt(deg), 0.0).astype(np.float32)

    NC_ = ncores
    NPC = _ru(N, NC_) // NC_
    NT = _ru(NPC, 128) // 128

    order = np.argsort(dst, kind='stable')
    s_s, s_d = src[order], dst[order]

    # per (core,tile) edge lists
    tiles = [[None] * NT for _ in range(NC_)]
    for core in range(NC_):
        for t in range(NT):
            lo = np.searchsorted(s_d, core * NPC + t * 128)
            hi = np.searchsorted(s_d, min(core * NPC + (t + 1) * 128,
                                          (core + 1) * NPC))
            tiles[core][t] = (s_s[lo:hi], s_d[lo:hi])

    TCT = max(max(_ru(len(tt[0]), 128) // 128 for tt in row) for row in tiles)
    TCT = max(_ru(TCT, 2), 2)
    if cfg is None:
        cfg = Cfg(N, E, H, C, G, OUT, TCT, NCORES=NC_)
        cfg.B1NZ = bool(np.any(np.asarray(b_gat) != 0))
        cfg.B2NZ = bool(np.any(np.asarray(b_gcn) != 0))
    assert cfg.TCT == TCT

    c = cfg
    # replicated tensors
    xT = np.zeros((C, c.NPAD), BF)
    xT[:, :N] = np.asarray(x).T.astype(BF)
    Wgf = np.asarray(W_gat).astype(np.float32)
    Wg = Wgf.astype(BF)
    Wg3 = Wgf.reshape(C, H, C)
    Mcat = np.zeros((C, 2 * H), BF)
    Mcat[:, 0:H] = np.einsum('khc,hc->kh', Wg3, np.asarray(att_src)).astype(BF)
    Mcat[:, H:2 * H] = np.einsum('khc,hc->kh', Wg3, np.asarray(att_dst)).astype(BF)
    bgat = np.zeros((128, c.DP), np.float32)
    bgat[:, :c.D1] = np.asarray(b_gat)[None, :]
    bgcn = np.zeros((128, c.DP), np.float32)
    bgcn[:, :c.D1] = np.asarray(b_gcn)[None, :]
    Wgcn = np.zeros((c.DP, c.DP), BF)
    Wgcn[:c.D1, :c.D1] = np.asarray(W_gcn).astype(BF)
    Wfc = np.zeros((2 * c.DP, OUT), np.float32)
    Wfc[0:c.D1] = np.asarray(W_fc)[0:c.D1]
    Wfc[c.DP:c.DP + c.D1] = np.asarray(W_fc)[c.D1:2 * c.D1]
    bfc = np.tile(np.asarray(b_fc).astype(np.float32)[None, :], (G, 1))
    cnt = np.bincount(np.asarray(batch), minlength=G).astype(np.float32)
    invcnt = np.tile((1.0 / np.maximum(cnt, 1.0))[None, :], (128, 1))

    batch_np = np.asarray(batch)
    shared = dict(xT=xT, Wg=Wg, Mcat=Mcat, Wgcn=Wgcn, Wfc=Wfc, bfc=bfc,
                  invcnt=invcnt, bgat=bgat, bgcn=bgcn)

    # y row index in the chunk-wise AllGathered layout, per source node id
    def yrow_of(j, core_of):
        local = j - core_of * NPC
        t = local // 128
        r = local % 128
        k = t // c.TPC
        return (k * c.NCORES * c.TPC * 128 + core_of * c.TPC * 128
                + (t - k * c.TPC) * 128 + r)

    in_maps = []
    for core in range(NC_):
        L = c.TC * 128
        sp = np.zeros(L, np.int64)
        dl = np.zeros(L, np.int64)
        valid = np.zeros(L, bool)
        for t in range(NT):
            ts, td = tiles[core][t]
            o = t * c.TCT * 128
            k = len(ts)
            sp[o:o + k] = ts
            dl[o:o + k] = td - (core * NPC + t * 128)
            valid[o:o + k] = True
        cs = sp // NPC
        yr = np.array([yrow_of(j, cj) for j, cj in zip(sp, cs)], np.int64)
        oh = np.zeros((c.TC, 128, 128), np.float32)
        ee = np.arange(L)
        oh[ee // 128, ee % 128, dl] = valid.astype(np.float32)
        # this core's dst-node ids per (tile, slot), clamped to valid rows
        dnids = np.zeros((NT, 128), np.int64)
        for t in range(NT):
            gids = core * NPC + t * 128 + np.arange(128)
            dnids[t] = np.minimum(gids, N - 1)
        sclm = np.zeros((128, NT), np.float32)
        for t in range(NT):
            gids = core * NPC + t * 128 + np.arange(128)
            ok = gids < min((core + 1) * NPC, N)
            sclm[ok, t] = dinv[gids[ok]]
        gonm = np.zeros((128, NT, G), BF)
        for t in range(NT):
            gids = core * NPC + t * 128 + np.arange(128)
            ok = gids < min((core + 1) * NPC, N)
            gonm[ok, t, batch_np[gids[ok]]] = 1.0
        m = dict(shared)
        m.update(
            sidx=_wrap_idx(sp), yidx=_wrap_idx(yr),
            dnid=_wrap_idx(dnids.reshape(-1)),
            ohb1=oh.transpose(1, 0, 2).astype(BF),
            ohb2=oh.transpose(1, 0, 2).astype(NPF8),
            ohT=oh.transpose(2, 0, 1).astype(BF),
            scl=sclm, gon=gonm)
        in_maps.append(m)
    return cfg, in_maps


_CACHE = {}


def run(inputs, trace=False):
    key = tuple(sorted((k, tuple(np.shape(v))) for k, v in inputs.items()))
    cfg, in_maps = preprocess(**inputs,
                              cfg=_CACHE[key][0] if key in _CACHE else None)
    if key not in _CACHE:
        _CACHE[key] = (cfg, build(cfg))
    cfg, nc = _CACHE[key]
    res = run_bass_kernel_spmd(nc, in_maps, core_ids=list(range(cfg.NCORES)),
                               trace=trace)
    return res.results[0]["out"].astype(np.float32), res


def kernel(**inputs):
    out, _ = run(inputs)
    return out


# revision 47
# speedup vs baseline: 1.3252x; 1.0516x over previous
"""GAT+GCN+pool GNN on 8 Trainium2 NeuronCores (Bass/Tile), fp8 edition.

Sharding: nodes/edges partitioned across 8 cores by destination-node range;
segment softmax and scatter-adds are core-local.  Per-edge row gathers use
dma_gather on fp8 rows (h stored as [2496 h | 32 a_src | 32 a_dst] fp8e4),
scatter-adds are DoubleRow fp8 one-hot matmuls (256 edges per pass).

GCN is computed as (A_hat x1) W (associativity) so the only big exchange is
an AllGather of the dinv-prescaled GAT output y = dinv*x1 in fp8 (26MB),
issued in chunks overlapped with phase-1 compute.  The same one-hot tensor
drives both scatter phases.  Dense GCN (bf16) runs per half-graph interleaved
with phase-2 scatter; graph pooling accumulates in PSUM across tiles.

Pipeline (per core, one NEFF):
  A)  h = x @ W_gat (bf16, replicated), a_src/a_dst folded matmul -> fp8 h_d
  1)  per dst-tile: gather fp8 rows per edge -> logits -> exp ->
      exp*h via DVE+GpSimd split -> DoubleRow one-hot scatter -> y (fp8)
  AG) chunked AllGather of y
  2)  per half: gather y rows, DoubleRow one-hot scatter -> agg; DMA-transpose;
      dense agg @ W_gcn (bf16) with fused relu*dinv; pooling matmul in PSUM
  AR) AllReduce pooled sums, gmean, FC, relu -> out [G, OUT]
"""

import sys
import os
import contextlib

if '/opt/trn_rl_repo' not in sys.path:
    sys.path.insert(0, '/opt/trn_rl_repo')

import numpy as np
import ml_dtypes

import concourse.bacc as bacc
import concourse.mybir as mybir
import concourse.tile as tile
from concourse.bass_utils import run_bass_kernel_spmd

F32 = mybir.dt.float32
BF16 = mybir.dt.bfloat16
F8 = mybir.dt.float8e4
I16 = mybir.dt.int16
BF = ml_dtypes.bfloat16
NPF8 = ml_dtypes.float8_e4m3
Alu = mybir.AluOpType
Act = mybir.ActivationFunctionType
DR = mybir.MatmulPerfMode.DoubleRow


def _ru(x, m):
    return (x + m - 1) // m * m


class Cfg:
    def __init__(self, N, E, H, C, G, OUT, TCT, NCORES=8, GRP=6, HD=22, AGC=1):
        self.N, self.E, self.H, self.C, self.G, self.OUT = N, E, H, C, G, OUT
        self.NCORES = NCORES
        self.D1 = H * C                              # 2496
        self.DP = _ru(self.D1 + 2 * H, 128)          # 2560 fp8 row bytes
        assert self.DP % 256 == 0
        self.NPC = _ru(N, NCORES) // NCORES          # nodes per core
        self.NT = _ru(self.NPC, 128) // 128          # dst tiles per core
        self.XWROWS = self.NT * 128
        self.XWFULL = NCORES * self.XWROWS
        self.ROWS_A = _ru(N, 128) // 128             # stage-A node tiles
        self.NPAD = self.ROWS_A * 128
        self.KS = self.DP // 128                     # dense k slabs
        self.FCK = 2 * self.KS
        assert TCT % 2 == 0
        self.TCT = TCT                               # chunks per dst tile
        self.TC = self.NT * TCT
        self.GRP = GRP                               # chunks per gather group
        assert GRP % 2 == 0
        self.NGRP = (TCT + GRP - 1) // GRP
        self.HD = HD                                 # heads multiplied on DVE
        self.AGC = AGC                               # allgather chunks
        assert self.NT % AGC == 0
        self.TPC = self.NT // AGC                    # tiles per AG chunk
        self.NHALF = 2                               # dense half-phases
        assert self.NT % self.NHALF == 0
        self.HT = self.NT // self.NHALF              # tiles per half
        self.B1NZ = False                            # b_gat nonzero
        self.B2NZ = False                            # b_gcn nonzero


def build(cfg):
    STAGE = int(os.environ.get("GNN_STAGE", "9"))
    DEBUG = int(os.environ.get("GNN_DEBUG", "0"))
    MV = int(os.environ.get("GNN_MV", "0"))
    CD = int(os.environ.get("GNN_CD", "4"))
    hd_env = os.environ.get("GNN_HD")
    if hd_env is not None:
        cfg.HD = int(hd_env)
    c = cfg
    nc = bacc.Bacc(None, target_bir_lowering=False)

    # ---- external inputs (replicated unless noted per-core) ----
    xT = nc.dram_tensor("xT", [c.C, c.NPAD], BF16, kind="ExternalInput")
    Wg = nc.dram_tensor("Wg", [c.C, c.D1], BF16, kind="ExternalInput")
    Mcat = nc.dram_tensor("Mcat", [c.C, 2 * c.H], BF16, kind="ExternalInput")
    Wgcn = nc.dram_tensor("Wgcn", [c.DP, c.DP], BF16, kind="ExternalInput")
    Wfc = nc.dram_tensor("Wfc", [2 * c.DP, c.OUT], F32, kind="ExternalInput")
    bfc = nc.dram_tensor("bfc", [c.G, c.OUT], F32, kind="ExternalInput")
    bgat = nc.dram_tensor("bgat", [128, c.DP], F32, kind="ExternalInput")
    bgcn = nc.dram_tensor("bgcn", [128, c.DP], F32, kind="ExternalInput")
    invcnt = nc.dram_tensor("invcnt", [128, c.G], F32, kind="ExternalInput")
    # per-core:
    sidx = nc.dram_tensor("sidx", [128, c.TC * 8], I16, kind="ExternalInput")
    yidx = nc.dram_tensor("yidx", [128, c.TC * 8], I16, kind="ExternalInput")
    dnid = nc.dram_tensor("dnid", [128, c.NT * 8], I16, kind="ExternalInput")
    ohb1 = nc.dram_tensor("ohb1", [128, c.TC, 128], BF16, kind="ExternalInput")
    ohb2 = nc.dram_tensor("ohb2", [128, c.TC, 128], F8, kind="ExternalInput")
    ohT = nc.dram_tensor("ohT", [128, c.TC, 128], BF16, kind="ExternalInput")
    scl = nc.dram_tensor("scl", [128, c.NT], F32, kind="ExternalInput")
    gon = nc.dram_tensor("gon", [128, c.NT, c.G], BF16, kind="ExternalInput")
    out = nc.dram_tensor("out", [c.G, c.OUT], F32, kind="ExternalOutput")
    if DEBUG:
        dbg_h = nc.dram_tensor("dbg_h", [c.NPAD, c.DP], BF16,
                               kind="ExternalOutput")
        dbg_y = nc.dram_tensor("dbg_y", [c.XWFULL, c.DP], mybir.dt.uint8,
                               kind="ExternalOutput")
        dbg_a = nc.dram_tensor("dbg_a", [c.XWROWS, c.DP], BF16,
                               kind="ExternalOutput")
        dbg_g = nc.dram_tensor("dbg_g", [128, c.KS * c.G], F32,
                               kind="ExternalOutput")
        dbg_x2 = nc.dram_tensor("dbg_x2", [c.XWROWS, c.DP], BF16,
                                kind="ExternalOutput")

    rg = [list(range(c.NCORES))]

    with tile.TileContext(nc) as tc:
        with (
            tc.tile_pool(name="dram", bufs=1, space="DRAM") as dram,
            tc.tile_pool(name="persist", bufs=1) as pp,
        ):
            h_d = dram.tile([c.NPAD, c.DP], BF16)
            y_d = dram.tile([c.XWROWS, c.DP], F8)
            yf_d = dram.tile([c.XWFULL, c.DP], F8, addr_space="Shared")
            aggb_d = dram.tile([c.XWROWS, c.DP], BF16)
            gs_in_d = dram.tile([128, c.KS * c.G], F32)
            gs_out_d = dram.tile([128, c.KS * c.G], F32, addr_space="Shared")

            # persistent smalls + resident GCN weights
            scl_sb = pp.tile([128, c.NT], F32)
            nc.sync.dma_start(scl_sb[:], scl[:])
            c02 = pp.tile([128, 1], BF16)
            nc.vector.memset(c02[:], 0.2)
            gon_sb = pp.tile([128, c.NT, c.G], BF16)
            nc.sync.dma_start(gon_sb[:], gon[:])
            wgcn_sb = pp.tile([128, c.KS, c.DP], BF16)
            for k in range(c.KS):
                nc.sync.dma_start(wgcn_sb[:, k, :],
                                  Wgcn[k * 128:(k + 1) * 128, :])
            if c.B1NZ:
                bgat_sb = pp.tile([128, c.DP], F32)
                nc.sync.dma_start(bgat_sb[:], bgat[:])
            if c.B2NZ:
                bgcn_sb = pp.tile([128, c.DP], F32)
                nc.sync.dma_start(bgcn_sb[:], bgcn[:])

            # ============ Stage A: h = x@Wg -> fp8 h_d with a-tail ============
            with tc.tile_pool(name="stageA", bufs=3) as sa, \
                 tc.tile_pool(name="stageAc", bufs=1) as sac, \
                 tc.tile_pool(name="psH", bufs=5, space="PSUM") as psH, \
                 tc.tile_pool(name="psHa", bufs=2, space="PSUM") as psHa:
                xT_sb = sac.tile([c.C, c.NPAD], BF16)
                nc.sync.dma_start(xT_sb[:], xT[:])
                Wg_sb = sac.tile([c.C, c.D1], BF16)
                nc.sync.dma_start(Wg_sb[:], Wg[:])
                Mc_sb = sac.tile([c.C, 2 * c.H], BF16)
                nc.sync.dma_start(Mc_sb[:], Mcat[:])
                for r in range(c.ROWS_A if STAGE >= 1 else 0):
                    lhs = xT_sb[:, r * 128:(r + 1) * 128]
                    hb = sa.tile([128, c.DP], BF16, tag="hb")
                    for i, j0 in enumerate(range(0, c.D1, 512)):
                        j1 = min(j0 + 512, c.D1)
                        ph = psH.tile([128, 512], F32, tag="ph")
                        nc.tensor.matmul(ph[:, 0:j1 - j0], lhs, Wg_sb[:, j0:j1],
                                         start=True, stop=True)
                        if i % 2 == 0:
                            nc.scalar.copy(hb[:, j0:j1], ph[:, 0:j1 - j0])
                        else:
                            nc.vector.tensor_copy(hb[:, j0:j1], ph[:, 0:j1 - j0])
                    pa = psHa.tile([128, 2 * c.H], F32, tag="pa")
                    nc.tensor.matmul(pa[:], lhs, Mc_sb[:], start=True, stop=True)
                    nc.vector.tensor_copy(hb[:, c.D1:c.D1 + 2 * c.H], pa[:])
                    nc.sync.dma_start(h_d[r * 128:(r + 1) * 128, :], hb[:])

            psA = contextlib.ExitStack()
            psA_pool = psA.enter_context(
                tc.tile_pool(name="psA", bufs=1, space="PSUM"))

            # ============ Phase 1: GAT edge softmax + scatter -> y ============
            with tc.tile_pool(name="p1", bufs=2) as p1, \
                 tc.tile_pool(name="p1h", bufs=2) as p1h, \
                 tc.tile_pool(name="p1o", bufs=2) as p1o, \
                 tc.tile_pool(name="psD", bufs=1, space="PSUM") as psD, \
                 tc.tile_pool(name="psE", bufs=2, space="PSUM") as psE:
                for t in range(c.NT if STAGE >= 2 else 0):
                    cs = t * c.TCT * 8
                    ce = (t + 1) * c.TCT * 8
                    si = p1.tile([128, c.TCT * 8], I16, tag="si")
                    nc.sync.dma_start(si[:], sidx[:, cs:ce])
                    ob = p1o.tile([128, c.TCT, 128], F8, tag="ob")
                    nc.sync.dma_start(ob[:], ohb2[:, t * c.TCT:(t + 1) * c.TCT, :])
                    obw = p1o.tile([128, c.TCT, 128], BF16, tag="obw")
                    nc.sync.dma_start(obw[:], ohb1[:, t * c.TCT:(t + 1) * c.TCT, :])
                    oT = p1o.tile([128, c.TCT, 128], BF16, tag="oT")
                    nc.sync.dma_start(oT[:], ohT[:, t * c.TCT:(t + 1) * c.TCT, :])
                    dn = p1.tile([128, 8], I16, tag="dn")
                    nc.sync.dma_start(dn[:], dnid[:, t * 8:(t + 1) * 8])
                    adt = p1.tile([128, 1, 128], BF16, tag="adt")
                    nc.gpsimd.dma_gather(adt[:], h_d[:, c.DP - 128:c.DP],
                                         dn[:], 128, 128, 128, elem_step=c.DP)

                    px = psA_pool.tile([128, c.DP], F32, tag="px")
                    pd = psD.tile([128, c.H], F32, tag="pd")
                    exf = p1.tile([128, c.TCT, c.H], BF16, tag="exf")
                    exf2 = p1.tile([128, c.TCT, c.H], BF16, tag="exf2")
                    ex8 = p1.tile([128, c.TCT, c.H], F8, tag="ex8")
                    for g in range(c.NGRP):
                        c0 = g * c.GRP
                        c1 = min(c0 + c.GRP, c.TCT)
                        nch = c1 - c0
                        hg = p1h.tile([128, c.GRP, c.DP], BF16, tag="hg")
                        nc.gpsimd.dma_gather(hg[:, 0:nch, :], h_d[:],
                                             si[:, c0 * 8:c1 * 8],
                                             nch * 128, nch * 128, c.DP)
                        peg = psE.tile([128, c.GRP, c.H], F32, tag="peg")
                        for ch in range(c0, c1):
                            nc.tensor.matmul(
                                peg[:, ch - c0, :], oT[:, ch, :],
                                adt[:, 0, 128 - c.H:128],
                                start=True, stop=True)
                        ev = exf[:, c0:c1, :]
                        nc.vector.tensor_add(ev, peg[:, 0:nch, :],
                                             hg[:, 0:nch, c.D1:c.D1 + c.H])
                        ev2 = exf2[:, c0:c1, :]
                        nc.vector.tensor_tensor(
                            ev2, ev,
                            c02[:, :, None].broadcast_to([128, nch, c.H]),
                            Alu.mult)
                        nc.vector.tensor_tensor(ev, ev, ev2, Alu.max)
                        nc.scalar.activation(ev, ev, Act.Exp)
                        nc.scalar.copy(ex8[:, c0:c1, :], ev)
                        mv = hg[:, 0:nch, 0:c.D1].rearrange(
                            "p t (h w) -> p t h w", h=c.H)
                        ebl = exf[:, c0:c1, 0:c.HD, None].broadcast_to(
                            [128, nch, c.HD, c.C])
                        ebp = exf[:, c0:c1, c.HD:c.H, None].broadcast_to(
                            [128, nch, c.H - c.HD, c.C])
                        nc.vector.tensor_mul(mv[:, :, 0:c.HD, :],
                                             mv[:, :, 0:c.HD, :], ebl)
                        if c.HD < c.H:
                            nc.gpsimd.tensor_mul(mv[:, :, c.HD:c.H, :],
                                                 mv[:, :, c.HD:c.H, :], ebp)
                        for ch2 in range(c0, c1, 2):
                            first = (ch2 == 0)
                            last = (ch2 == c.TCT - 2)
                            nc.tensor.matmul(pd[:], ob[:, ch2:ch2 + 2, :],
                                             ex8[:, ch2:ch2 + 2, :],
                                             start=first, stop=last,
                                             perf_mode=DR)
                        for ch in range(c0, c1):
                            for j0 in range(0, c.DP, 512):
                                nc.tensor.matmul(
                                    px[:, j0:j0 + 512], obw[:, ch, :],
                                    hg[:, ch - c0, j0:j0 + 512],
                                    start=(ch == 0), stop=(ch == c.TCT - 1))
                    rdn = p1.tile([128, c.H], F32, tag="rdn")
                    nc.vector.reciprocal(rdn[:], pd[:])
                    sc = p1.tile([128, c.H], F32, tag="sc")
                    nc.vector.tensor_mul(
                        sc[:], rdn[:],
                        scl_sb[:, t:t + 1].broadcast_to([128, c.H]))
                    yt = p1.tile([128, c.DP], F8, tag="yt")
                    nc.vector.memset(yt[:, c.D1:], 0.0)
                    pxv = px[:, 0:c.D1].rearrange("p (h w) -> p h w", h=c.H)
                    ytv = yt[:, 0:c.D1].rearrange("p (h w) -> p h w", h=c.H)
                    scb = sc[:, :, None].broadcast_to([128, c.H, c.C])
                    if not c.B1NZ:
                        nc.vector.scalar_tensor_tensor(ytv, pxv, 0.0, scb,
                                                       Alu.max, Alu.mult)
                    else:
                        x1f = p1.tile([128, c.D1], F32, tag="x1f")
                        x1v = x1f[:].rearrange("p (h w) -> p h w", h=c.H)
                        rb = rdn[:, :, None].broadcast_to([128, c.H, c.C])
                        nc.vector.tensor_mul(x1v, pxv, rb)
                        nc.vector.tensor_add(x1f[:], x1f[:],
                                             bgat_sb[:, 0:c.D1])
                        nc.vector.tensor_scalar_max(x1f[:], x1f[:], 0.0)
                        dvb = scl_sb[:, t:t + 1].broadcast_to([128, c.D1])
                        nc.vector.tensor_tensor(yt[:, 0:c.D1], x1f[:], dvb,
                                                Alu.mult)
                    nc.sync.dma_start(y_d[t * 128:(t + 1) * 128, :], yt[:])
                    # chunked AllGather as soon as a chunk's tiles are done
                    if STAGE >= 3 and (t + 1) % c.TPC == 0:
                        k = (t + 1) // c.TPC - 1
                        r0 = k * c.TPC * 128
                        r1 = (k + 1) * c.TPC * 128
                        nc.gpsimd.collective_compute(
                            "AllGather", Alu.bypass,
                            ins=[y_d[r0:r1, :]],
                            outs=[yf_d[r0 * c.NCORES:r1 * c.NCORES, :]],
                            replica_groups=rg)

            if DEBUG:
                nc.sync.dma_start(dbg_h[:], h_d[:])
                nc.sync.dma_start(dbg_y[:], yf_d[:].bitcast(mybir.dt.uint8))

            # ============ Phase 2: GCN scatter + dense + pooling ============
            with tc.tile_pool(name="p2", bufs=2) as p2, \
                 tc.tile_pool(name="p2h", bufs=3) as p2h, \
                 tc.tile_pool(name="p2o", bufs=2) as p2o, \
                 tc.tile_pool(name="gd", bufs=1) as gd, \
                 tc.tile_pool(name="gw", bufs=2) as gw, \
                 tc.tile_pool(name="psW", bufs=2, space="PSUM") as psW, \
                 tc.tile_pool(name="psP", bufs=1, space="PSUM") as psP:
                gacc = pp.tile([128, c.KS * c.G], F32)
                nc.vector.memset(gacc[:], 0.0)

                for hf in range(c.NHALF if STAGE >= 4 else 0):
                    for t in range(hf * c.HT, (hf + 1) * c.HT):
                        cs = t * c.TCT * 8
                        ce = (t + 1) * c.TCT * 8
                        xi = p2.tile([128, c.TCT * 8], I16, tag="xi")
                        nc.sync.dma_start(xi[:], yidx[:, cs:ce])
                        ob2 = p2o.tile([128, c.TCT, 128], F8, tag="ob2")
                        nc.sync.dma_start(ob2[:],
                                          ohb2[:, t * c.TCT:(t + 1) * c.TCT, :])
                        px2 = psA_pool.tile([128, c.DP], F32, tag="px")
                        for g in range(c.NGRP):
                            c0 = g * c.GRP
                            c1 = min(c0 + c.GRP, c.TCT)
                            nch = c1 - c0
                            yg = p2h.tile([128, c.GRP, c.DP], F8, tag="hg")
                            nc.gpsimd.dma_gather(yg[:, 0:nch, :], yf_d[:],
                                                 xi[:, c0 * 8:c1 * 8],
                                                 nch * 128, nch * 128, c.DP)
                            for ch2 in range(c0, c1, 2):
                                first = (ch2 == 0)
                                last = (ch2 == c.TCT - 2)
                                for j0 in range(0, c.DP, 512):
                                    nc.tensor.matmul(
                                        px2[:, j0:j0 + 512],
                                        ob2[:, ch2:ch2 + 2, :],
                                        yg[:, ch2 - c0:ch2 - c0 + 2,
                                           j0:j0 + 512],
                                        start=first, stop=last, perf_mode=DR)
                        agt = p2.tile([128, c.DP], BF16, tag="agt")
                        nc.scalar.copy(agt[:], px2[:])
                        nc.sync.dma_start(aggb_d[t * 128:(t + 1) * 128, :],
                                          agt[:])
                    if STAGE < 5:
                        continue
                    # dense for this half: transpose agg, matmul, relu*dinv
                    hr0 = hf * c.HT * 128
                    hr1 = (hf + 1) * c.HT * 128
                    aggT = gd.tile([128, c.KS, c.HT * 128], BF16, tag="aT")
                    for k in range(c.KS):
                        nc.sync.dma_start(aggT[:, k, :],
                                          aggb_d[hr0:hr1, k * 128:(k + 1) * 128],
                                          transpose=True)
                    for m in range(hf * c.HT, (hf + 1) * c.HT):
                        mo = (m - hf * c.HT) * 128
                        xt2 = gw.tile([128, c.DP], BF16, tag="xt2")
                        for j0 in range(0, c.DP, 512):
                            pw = psW.tile([128, 512], F32, tag="pw")
                            for k in range(c.KS):
                                nc.tensor.matmul(
                                    pw[:],
                                    aggT[:, k, mo:mo + 128],
                                    wgcn_sb[:, k, j0:j0 + 512],
                                    start=(k == 0), stop=(k == c.KS - 1))
                            if not c.B2NZ:
                                nc.scalar.activation(
                                    xt2[:, j0:j0 + 512], pw[:],
                                    Act.Relu, scale=scl_sb[:, m:m + 1])
                            else:
                                xf = gw.tile([128, 512], F32, tag="xf")
                                dvb = scl_sb[:, m:m + 1].broadcast_to(
                                    [128, 512])
                                nc.vector.tensor_tensor(
                                    xf[:], pw[:], dvb, Alu.mult)
                                nc.vector.tensor_add(
                                    xf[:], xf[:], bgcn_sb[:, j0:j0 + 512])
                                nc.vector.tensor_scalar_max(
                                    xt2[:, j0:j0 + 512], xf[:], 0.0)
                        if DEBUG:
                            nc.sync.dma_start(
                                dbg_x2[m * 128:(m + 1) * 128, :], xt2[:])
                        for fb in range(0, c.KS, 8):
                            fe = min(fb + 8, c.KS)
                            pgt = psP.tile([128, 8, c.G], F32, tag="pg")
                            for fs in range(fb, fe):
                                nc.tensor.matmul(
                                    pgt[:, fs - fb, :],
                                    xt2[:, fs * 128:(fs + 1) * 128],
                                    gon_sb[:, m, :],
                                    start=True, stop=True)
                            nc.vector.tensor_add(
                                gacc[:, fb * c.G:fe * c.G],
                                gacc[:, fb * c.G:fe * c.G],
                                pgt[:, 0:fe - fb, :].rearrange(
                                    "p k g -> p (k g)"))
                nc.gpsimd.dma_start(gs_in_d[:], gacc[:])
                if DEBUG:
                    nc.sync.dma_start(dbg_a[:], aggb_d[:])
                    nc.sync.dma_start(dbg_g[:], gs_in_d[:])
            psA.close()

            # ============ AllReduce pooled sums + FC ============
            if STAGE >= 6:
                nc.gpsimd.collective_compute(
                    "AllReduce", Alu.add, ins=[gs_in_d[:]], outs=[gs_out_d[:]],
                    replica_groups=rg)
            with tc.tile_pool(name="fc", bufs=1) as fc, \
                 tc.tile_pool(name="psS", bufs=1, space="PSUM") as psS:
              if STAGE < 6:
                dz = fc.tile([c.G, c.OUT], F32)
                nc.vector.memset(dz[:], 0.0)
                nc.sync.dma_start(out[:], dz[:])
              else:
                gsar = fc.tile([128, c.KS, c.G], F32)
                nc.sync.dma_start(gsar[:],
                                  gs_out_d[:].rearrange("p (k g) -> p k g",
                                                        k=c.KS))
                iv_sb = fc.tile([128, c.G], F32)
                nc.sync.dma_start(iv_sb[:], invcnt[:])
                gm = fc.tile([128, c.KS, c.G], F32)
                nc.vector.tensor_mul(
                    gm[:], gsar[:],
                    iv_sb[:, None, :].broadcast_to([128, c.KS, c.G]))
                wf_sb = fc.tile([128, c.FCK, c.OUT], F32)
                nc.sync.dma_start(
                    wf_sb[:], Wfc[:].rearrange("(k p) o -> p k o", p=128))
                pf = psS.tile([c.G, c.OUT], F32, tag="sm")
                for k in range(c.FCK):
                    lhs = gm[:, k, :] if k < c.KS else gsar[:, k - c.KS, :]
                    nc.tensor.matmul(pf[:], lhs, wf_sb[:, k, :],
                                     start=(k == 0), stop=(k == c.FCK - 1))
                bf_sb = fc.tile([c.G, c.OUT], F32)
                nc.sync.dma_start(bf_sb[:], bfc[:])
                ot = fc.tile([c.G, c.OUT], F32)
                nc.vector.tensor_add(ot[:], pf[:], bf_sb[:])
                nc.vector.tensor_scalar_max(ot[:], ot[:], 0.0)
                nc.sync.dma_start(out[:], ot[:])

    nc.compile()
    return nc


# ================= host-side preprocessing =================

def _wrap_idx(a):
    """[L] int -> [128, L//16] int16 wrapped (i -> [i%16, i//16]) + 8x repl."""
    w = a.reshape(-1, 16).T.astype(np.int16)
    return np.tile(w, (8, 1)).copy()


def preprocess(x, edge_index, batch, num_graphs, W_gat, att_src, att_dst,
               b_gat, W_gcn, b_gcn, W_fc, b_fc, cfg=None, ncores=8):
    N, C = x.shape
    E = edge_index.shape[1]
    H = att_src.shape[0]
    G = int(num_graphs)
    OUT = W_fc.shape[1]

    src = np.concatenate([np.asarray(edge_index[0]), np.arange(N)]).astype(np.int64)
    dst = np.concatenate([np.asarray(edge_index[1]), np.arange(N)]).astype(np.int64)
    deg = np.bincount(dst, minlength=N).astype(np.float32)
    dinv = np.where(deg > 0, 1.0 / np.sqrt(deg), 0.0).astype(np.float32)

    NC_ = ncores
    NPC = _ru(N, NC_) // NC_
    NT = _ru(NPC, 128) // 128

    order = np.argsort(dst, kind='stable')
    s_s, s_d = src[order], dst[order]

    # per (core,tile) edge lists
    tiles = [[None] * NT for _ in range(NC_)]
    for core in range(NC_):
        for t in range(NT):
            lo = np.searchsorted(s_d, core * NPC + t * 128)
            hi = np.searchsorted(s_d, min(core * NPC + (t + 1) * 128,
                                          (core + 1) * NPC))
            tiles[core][t] = (s_s[lo:hi], s_d[lo:hi])

    TCT = max(max(_ru(len(tt[0]), 128) // 128 for tt in row) for row in tiles)
    TCT = max(_ru(TCT, 2), 2)
    if cfg is None:
        cfg = Cfg(N, E, H, C, G, OUT, TCT, NCORES=NC_)
        cfg.B1NZ = bool(np.any(np.asarray(b_gat) != 0))
        cfg.B2NZ = bool(np.any(np.asarray(b_gcn) != 0))
    assert cfg.TCT == TCT

    c = cfg
    # replicated tensors
    xT = np.zeros((C, c.NPAD), BF)
    xT[:, :N] = np.asarray(x).T.astype(BF)
    Wgf = np.asarray(W_gat).astype(np.float32)
    Wg = Wgf.astype(BF)
    Wg3 = Wgf.reshape(C, H, C)
    Mcat = np.zeros((C, 2 * H), BF)
    Mcat[:, 0:H] = np.einsum('khc,hc->kh', Wg3, np.asarray(att_src)).astype(BF)
    Mcat[:, H:2 * H] = np.einsum('khc,hc->kh', Wg3, np.asarray(att_dst)).astype(BF)
    bgat = np.zeros((128, c.DP), np.float32)
    bgat[:, :c.D1] = np.asarray(b_gat)[None, :]
    bgcn = np.zeros((128, c.DP), np.float32)
    bgcn[:, :c.D1] = np.asarray(b_gcn)[None, :]
    Wgcn = np.zeros((c.DP, c.DP), BF)
    Wgcn[:c.D1, :c.D1] = np.asarray(W_gcn).astype(BF)
    Wfc = np.zeros((2 * c.DP, OUT), np.float32)
    Wfc[0:c.D1] = np.asarray(W_fc)[0:c.D1]
    Wfc[c.DP:c.DP + c.D1] = np.asarray(W_fc)[c.D1:2 * c.D1]
    bfc = np.tile(np.asarray(b_fc).astype(np.float32)[None, :], (G, 1))
    cnt = np.bincount(np.asarray(batch), minlength=G).astype(np.float32)
    invcnt = np.tile((1.0 / np.maximum(cnt, 1.0))[None, :], (128, 1))

    batch_np = np.asarray(batch)
    shared = dict(xT=xT, Wg=Wg, Mcat=Mcat, Wgcn=Wgcn, Wfc=Wfc, bfc=bfc,
                  invcnt=invcnt, bgat=bgat, bgcn=bgcn)

    # y row index in the chunk-wise AllGathered layout, per source node id
    def yrow_of(j, core_of):
        local = j - core_of * NPC
        t = local // 128
        r = local % 128
        k = t // c.TPC
        return (k * c.NCORES * c.TPC * 128 + core_of * c.TPC * 128
                + (t - k * c.TPC) * 128 + r)

    in_maps = []
    for core in range(NC_):
        L = c.TC * 128
        sp = np.zeros(L, np.int64)
        dl = np.zeros(L, np.int64)
        valid = np.zeros(L, bool)
        for t in range(NT):
            ts, td = tiles[core][t]
            o = t * c.TCT * 128
            k = len(ts)
            sp[o:o + k] = ts
            dl[o:o + k] = td - (core * NPC + t * 128)
            valid[o:o + k] = True
        cs = sp // NPC
        yr = np.array([yrow_of(j, cj) for j, cj in zip(sp, cs)], np.int64)
        oh = np.zeros((c.TC, 128, 128), np.float32)
        ee = np.arange(L)
        oh[ee // 128, ee % 128, dl] = valid.astype(np.float32)
        # this core's dst-node ids per (tile, slot), clamped to valid rows
        dnids = np.zeros((NT, 128), np.int64)
        for t in range(NT):
            gids = core * NPC + t * 128 + np.arange(128)
            dnids[t] = np.minimum(gids, N - 1)
        sclm = np.zeros((128, NT), np.float32)
        for t in range(NT):
            gids = core * NPC + t * 128 + np.arange(128)
            ok = gids < min((core + 1) * NPC, N)
            sclm[ok, t] = dinv[gids[ok]]
        gonm = np.zeros((128, NT, G), BF)
        for t in range(NT):
            gids = core * NPC + t * 128 + np.arange(128)
            ok = gids < min((core + 1) * NPC, N)
            gonm[ok, t, batch_np[gids[ok]]] = 1.0
        m = dict(shared)
        m.update(
            sidx=_wrap_idx(sp), yidx=_wrap_idx(yr),
            dnid=_wrap_idx(dnids.reshape(-1)),
            ohb1=oh.transpose(1, 0, 2).astype(BF),
            ohb2=oh.transpose(1, 0, 2).astype(NPF8),
            ohT=oh.transpose(2, 0, 1).astype(BF),
            scl=sclm, gon=gonm)
        in_maps.append(m)
    return cfg, in_maps


_CACHE = {}


def run(inputs, trace=False):
    key = tuple(sorted((k, tuple(np.shape(v))) for k, v in inputs.items()))
    cfg, in_maps = preprocess(**inputs,
                              cfg=_CACHE[key][0] if key in _CACHE else None)
    if key not in _CACHE:
        _CACHE[key] = (cfg, build(cfg))
    cfg, nc = _CACHE[key]
    res = run_bass_kernel_spmd(nc, in_maps, core_ids=list(range(cfg.NCORES)),
                               trace=trace)
    return res.results[0]["out"].astype(np.float32), res


def kernel(**inputs):
    out, _ = run(inputs)
    return out


# revision 50
# speedup vs baseline: 1.3286x; 1.0026x over previous
"""GAT+GCN+pool GNN on 8 Trainium2 NeuronCores (Bass/Tile), fp8 edition.

Sharding: nodes/edges partitioned across 8 cores by destination-node range;
segment softmax and scatter-adds are core-local.  Per-edge row gathers use
dma_gather on fp8 rows (h stored as [2496 h | 32 a_src | 32 a_dst] fp8e4),
scatter-adds are DoubleRow fp8 one-hot matmuls (256 edges per pass).

GCN is computed as (A_hat x1) W (associativity) so the only big exchange is
an AllGather of the dinv-prescaled GAT output y = dinv*x1 in fp8 (26MB),
issued in chunks overlapped with phase-1 compute.  The same one-hot tensor
drives both scatter phases.  Dense GCN (bf16) runs per half-graph interleaved
with phase-2 scatter; graph pooling accumulates in PSUM across tiles.

Pipeline (per core, one NEFF):
  A)  h = x @ W_gat (bf16, replicated), a_src/a_dst folded matmul -> fp8 h_d
  1)  per dst-tile: gather fp8 rows per edge -> logits -> exp ->
      exp*h via DVE+GpSimd split -> DoubleRow one-hot scatter -> y (fp8)
  AG) chunked AllGather of y
  2)  per half: gather y rows, DoubleRow one-hot scatter -> agg; DMA-transpose;
      dense agg @ W_gcn (bf16) with fused relu*dinv; pooling matmul in PSUM
  AR) AllReduce pooled sums, gmean, FC, relu -> out [G, OUT]
"""

import sys
import os
import contextlib

if '/opt/trn_rl_repo' not in sys.path:
    sys.path.insert(0, '/opt/trn_rl_repo')

import numpy as np
import ml_dtypes

import concourse.bacc as bacc
import concourse.mybir as mybir
import concourse.tile as tile
from concourse.bass_utils import run_bass_kernel_spmd

F32 = mybir.dt.float32
BF16 = mybir.dt.bfloat16
F8 = mybir.dt.float8e4
I16 = mybir.dt.int16
BF = ml_dtypes.bfloat16
NPF8 = ml_dtypes.float8_e4m3
Alu = mybir.AluOpType
Act = mybir.ActivationFunctionType
DR = mybir.MatmulPerfMode.DoubleRow


def _ru(x, m):
    return (x + m - 1) // m * m


class Cfg:
    def __init__(self, N, E, H, C, G, OUT, TCT, NCORES=8, GRP=6, HD=22, AGC=1):
        self.N, self.E, self.H, self.C, self.G, self.OUT = N, E, H, C, G, OUT
        self.NCORES = NCORES
        self.D1 = H * C                              # 2496
        self.DP = _ru(self.D1 + 2 * H, 128)          # 2560 fp8 row bytes
        assert self.DP % 256 == 0
        self.NPC = _ru(N, NCORES) // NCORES          # nodes per core
        self.NT = _ru(self.NPC, 128) // 128          # dst tiles per core
        self.XWROWS = self.NT * 128
        self.XWFULL = NCORES * self.XWROWS
        self.ROWS_A = _ru(N, 128) // 128             # stage-A node tiles
        self.NPAD = self.ROWS_A * 128
        self.KS = self.DP // 128                     # dense k slabs
        self.FCK = 2 * self.KS
        assert TCT % 2 == 0
        self.TCT = TCT                               # chunks per dst tile
        self.TC = self.NT * TCT
        self.GRP = GRP                               # chunks per gather group
        assert GRP % 2 == 0
        self.NGRP = (TCT + GRP - 1) // GRP
        self.HD = HD                                 # heads multiplied on DVE
        self.AGC = AGC                               # allgather chunks
        assert self.NT % AGC == 0
        self.TPC = self.NT // AGC                    # tiles per AG chunk
        self.NHALF = 2                               # dense half-phases
        assert self.NT % self.NHALF == 0
        self.HT = self.NT // self.NHALF              # tiles per half
        self.B1NZ = False                            # b_gat nonzero
        self.B2NZ = False                            # b_gcn nonzero


def build(cfg):
    STAGE = int(os.environ.get("GNN_STAGE", "9"))
    DEBUG = int(os.environ.get("GNN_DEBUG", "0"))
    MV = int(os.environ.get("GNN_MV", "0"))
    CD = int(os.environ.get("GNN_CD", "4"))
    hd_env = os.environ.get("GNN_HD")
    if hd_env is not None:
        cfg.HD = int(hd_env)
    c = cfg
    nc = bacc.Bacc(None, target_bir_lowering=False)

    # ---- external inputs (replicated unless noted per-core) ----
    xT = nc.dram_tensor("xT", [c.C, c.NPAD], BF16, kind="ExternalInput")
    Wg = nc.dram_tensor("Wg", [c.C, c.D1], BF16, kind="ExternalInput")
    Mcat = nc.dram_tensor("Mcat", [c.C, 2 * c.H], BF16, kind="ExternalInput")
    Wgcn = nc.dram_tensor("Wgcn", [c.DP, c.DP], BF16, kind="ExternalInput")
    Wfc = nc.dram_tensor("Wfc", [2 * c.DP, c.OUT], F32, kind="ExternalInput")
    bfc = nc.dram_tensor("bfc", [c.G, c.OUT], F32, kind="ExternalInput")
    bgat = nc.dram_tensor("bgat", [128, c.DP], F32, kind="ExternalInput")
    bgcn = nc.dram_tensor("bgcn", [128, c.DP], F32, kind="ExternalInput")
    invcnt = nc.dram_tensor("invcnt", [128, c.G], F32, kind="ExternalInput")
    # per-core:
    sidx = nc.dram_tensor("sidx", [128, c.TC * 8], I16, kind="ExternalInput")
    yidx = nc.dram_tensor("yidx", [128, c.TC * 8], I16, kind="ExternalInput")
    dnid = nc.dram_tensor("dnid", [128, c.NT * 8], I16, kind="ExternalInput")
    ohb1 = nc.dram_tensor("ohb1", [128, c.TC, 128], BF16, kind="ExternalInput")
    ohb2 = nc.dram_tensor("ohb2", [128, c.TC, 128], F8, kind="ExternalInput")
    ohT = nc.dram_tensor("ohT", [128, c.TC, 128], BF16, kind="ExternalInput")
    scl = nc.dram_tensor("scl", [128, c.NT], F32, kind="ExternalInput")
    gon = nc.dram_tensor("gon", [128, c.NT, c.G], BF16, kind="ExternalInput")
    out = nc.dram_tensor("out", [c.G, c.OUT], F32, kind="ExternalOutput")
    if DEBUG:
        dbg_h = nc.dram_tensor("dbg_h", [c.NPAD, c.DP], BF16,
                               kind="ExternalOutput")
        dbg_y = nc.dram_tensor("dbg_y", [c.XWFULL, c.DP], mybir.dt.uint8,
                               kind="ExternalOutput")
        dbg_a = nc.dram_tensor("dbg_a", [c.XWROWS, c.DP], BF16,
                               kind="ExternalOutput")
        dbg_g = nc.dram_tensor("dbg_g", [128, c.KS * c.G], F32,
                               kind="ExternalOutput")
        dbg_x2 = nc.dram_tensor("dbg_x2", [c.XWROWS, c.DP], BF16,
                                kind="ExternalOutput")

    rg = [list(range(c.NCORES))]

    with tile.TileContext(nc) as tc:
        with (
            tc.tile_pool(name="dram", bufs=1, space="DRAM") as dram,
            tc.tile_pool(name="persist", bufs=1) as pp,
        ):
            h_d = dram.tile([c.NPAD, c.DP], BF16)
            y_d = dram.tile([c.XWROWS, c.DP], F8)
            yf_d = dram.tile([c.XWFULL, c.DP], F8, addr_space="Shared")
            aggb_d = dram.tile([c.XWROWS, c.DP], BF16)
            gs_in_d = dram.tile([128, c.KS * c.G], F32)
            gs_out_d = dram.tile([128, c.KS * c.G], F32, addr_space="Shared")

            # persistent smalls + resident GCN weights
            scl_sb = pp.tile([128, c.NT], F32)
            nc.sync.dma_start(scl_sb[:], scl[:])
            c02 = pp.tile([128, 1], BF16)
            nc.vector.memset(c02[:], 0.2)
            gon_sb = pp.tile([128, c.NT, c.G], BF16)
            nc.sync.dma_start(gon_sb[:], gon[:])
            wgcn_sb = pp.tile([128, c.KS, c.DP], BF16)
            for k in range(c.KS):
                nc.sync.dma_start(wgcn_sb[:, k, :],
                                  Wgcn[k * 128:(k + 1) * 128, :])
            if c.B1NZ:
                bgat_sb = pp.tile([128, c.DP], F32)
                nc.sync.dma_start(bgat_sb[:], bgat[:])
            if c.B2NZ:
                bgcn_sb = pp.tile([128, c.DP], F32)
                nc.sync.dma_start(bgcn_sb[:], bgcn[:])

            # ============ Stage A: h = x@Wg -> fp8 h_d with a-tail ============
            with tc.tile_pool(name="stageA", bufs=3) as sa, \
                 tc.tile_pool(name="stageAc", bufs=1) as sac, \
                 tc.tile_pool(name="psH", bufs=5, space="PSUM") as psH, \
                 tc.tile_pool(name="psHa", bufs=2, space="PSUM") as psHa:
                xT_sb = sac.tile([c.C, c.NPAD], BF16)
                nc.sync.dma_start(xT_sb[:], xT[:])
                Wg_sb = sac.tile([c.C, c.D1], BF16)
                nc.sync.dma_start(Wg_sb[:], Wg[:])
                Mc_sb = sac.tile([c.C, 2 * c.H], BF16)
                nc.sync.dma_start(Mc_sb[:], Mcat[:])
                for r in range(c.ROWS_A if STAGE >= 1 else 0):
                    lhs = xT_sb[:, r * 128:(r + 1) * 128]
                    hb = sa.tile([128, c.DP], BF16, tag="hb")
                    for i, j0 in enumerate(range(0, c.D1, 512)):
                        j1 = min(j0 + 512, c.D1)
                        ph = psH.tile([128, 512], F32, tag="ph")
                        nc.tensor.matmul(ph[:, 0:j1 - j0], lhs, Wg_sb[:, j0:j1],
                                         start=True, stop=True)
                        if i % 2 == 0:
                            nc.scalar.copy(hb[:, j0:j1], ph[:, 0:j1 - j0])
                        else:
                            nc.vector.tensor_copy(hb[:, j0:j1], ph[:, 0:j1 - j0])
                    pa = psHa.tile([128, 2 * c.H], F32, tag="pa")
                    nc.tensor.matmul(pa[:], lhs, Mc_sb[:], start=True, stop=True)
                    nc.vector.tensor_copy(hb[:, c.D1:c.D1 + 2 * c.H], pa[:])
                    nc.sync.dma_start(h_d[r * 128:(r + 1) * 128, :], hb[:])

            psA = contextlib.ExitStack()
            psA_pool = psA.enter_context(
                tc.tile_pool(name="psA", bufs=1, space="PSUM"))

            # ============ Phase 1: GAT edge softmax + scatter -> y ============
            with tc.tile_pool(name="p1", bufs=2) as p1, \
                 tc.tile_pool(name="p1h", bufs=2) as p1h, \
                 tc.tile_pool(name="p1o", bufs=2) as p1o, \
                 tc.tile_pool(name="psD", bufs=1, space="PSUM") as psD, \
                 tc.tile_pool(name="psE", bufs=2, space="PSUM") as psE:
                for t in range(c.NT if STAGE >= 2 else 0):
                    cs = t * c.TCT * 8
                    ce = (t + 1) * c.TCT * 8
                    si = p1.tile([128, c.TCT * 8], I16, tag="si")
                    nc.sync.dma_start(si[:], sidx[:, cs:ce])
                    ob = p1o.tile([128, c.TCT, 128], F8, tag="ob")
                    nc.sync.dma_start(ob[:], ohb2[:, t * c.TCT:(t + 1) * c.TCT, :])
                    obw = p1o.tile([128, c.TCT, 128], BF16, tag="obw")
                    nc.sync.dma_start(obw[:], ohb1[:, t * c.TCT:(t + 1) * c.TCT, :])
                    oT = p1o.tile([128, c.TCT, 128], BF16, tag="oT")
                    nc.sync.dma_start(oT[:], ohT[:, t * c.TCT:(t + 1) * c.TCT, :])
                    dn = p1.tile([128, 8], I16, tag="dn")
                    nc.sync.dma_start(dn[:], dnid[:, t * 8:(t + 1) * 8])
                    adt = p1.tile([128, 1, 128], BF16, tag="adt")
                    nc.gpsimd.dma_gather(adt[:], h_d[:, c.DP - 128:c.DP],
                                         dn[:], 128, 128, 128, elem_step=c.DP)

                    px = psA_pool.tile([128, c.DP], F32, tag="px")
                    pd = psD.tile([128, c.H], F32, tag="pd")
                    exf = p1.tile([128, c.TCT, c.H], BF16, tag="exf")
                    exf2 = p1.tile([128, c.TCT, c.H], BF16, tag="exf2")
                    ex8 = p1.tile([128, c.TCT, c.H], F8, tag="ex8")
                    for g in range(c.NGRP):
                        c0 = g * c.GRP
                        c1 = min(c0 + c.GRP, c.TCT)
                        nch = c1 - c0
                        hg = p1h.tile([128, c.GRP, c.DP], BF16, tag="hg")
                        nc.gpsimd.dma_gather(hg[:, 0:nch, :], h_d[:],
                                             si[:, c0 * 8:c1 * 8],
                                             nch * 128, nch * 128, c.DP)
                        peg = psE.tile([128, c.GRP, c.H], F32, tag="peg")
                        for ch in range(c0, c1):
                            nc.tensor.matmul(
                                peg[:, ch - c0, :], oT[:, ch, :],
                                adt[:, 0, 128 - c.H:128],
                                start=True, stop=True)
                        ev = exf[:, c0:c1, :]
                        nc.vector.tensor_add(ev, peg[:, 0:nch, :],
                                             hg[:, 0:nch, c.D1:c.D1 + c.H])
                        ev2 = exf2[:, c0:c1, :]
                        nc.vector.tensor_tensor(
                            ev2, ev,
                            c02[:, :, None].broadcast_to([128, nch, c.H]),
                            Alu.mult)
                        nc.vector.tensor_tensor(ev, ev, ev2, Alu.max)
                        nc.scalar.activation(ev, ev, Act.Exp)
                        nc.scalar.copy(ex8[:, c0:c1, :], ev)
                        mv = hg[:, 0:nch, 0:c.D1].rearrange(
                            "p t (h w) -> p t h w", h=c.H)
                        ebl = exf[:, c0:c1, 0:c.HD, None].broadcast_to(
                            [128, nch, c.HD, c.C])
                        ebp = exf[:, c0:c1, c.HD:c.H, None].broadcast_to(
                            [128, nch, c.H - c.HD, c.C])
                        nc.vector.tensor_mul(mv[:, :, 0:c.HD, :],
                                             mv[:, :, 0:c.HD, :], ebl)
                        if c.HD < c.H:
                            nc.gpsimd.tensor_mul(mv[:, :, c.HD:c.H, :],
                                                 mv[:, :, c.HD:c.H, :], ebp)
                        for ch2 in range(c0, c1, 2):
                            first = (ch2 == 0)
                            last = (ch2 == c.TCT - 2)
                            nc.tensor.matmul(pd[:], ob[:, ch2:ch2 + 2, :],
                                             ex8[:, ch2:ch2 + 2, :],
                                             start=first, stop=last,
                                             perf_mode=DR)
                        for ch in range(c0, c1):
                            for j0 in range(0, c.DP, 512):
                                nc.tensor.matmul(
                                    px[:, j0:j0 + 512], obw[:, ch, :],
                                    hg[:, ch - c0, j0:j0 + 512],
                                    start=(ch == 0), stop=(ch == c.TCT - 1))
                    rdn = p1.tile([128, c.H], F32, tag="rdn")
                    nc.vector.reciprocal(rdn[:], pd[:])
                    sc = p1.tile([128, c.H], F32, tag="sc")
                    nc.vector.tensor_mul(
                        sc[:], rdn[:],
                        scl_sb[:, t:t + 1].broadcast_to([128, c.H]))
                    yt = p1.tile([128, c.DP], F8, tag="yt")
                    nc.vector.memset(yt[:, c.D1:], 0.0)
                    pxv = px[:, 0:c.D1].rearrange("p (h w) -> p h w", h=c.H)
                    ytv = yt[:, 0:c.D1].rearrange("p (h w) -> p h w", h=c.H)
                    scb = sc[:, :, None].broadcast_to([128, c.H, c.C])
                    if not c.B1NZ:
                        nc.vector.scalar_tensor_tensor(ytv, pxv, 0.0, scb,
                                                       Alu.max, Alu.mult)
                    else:
                        x1f = p1.tile([128, c.D1], F32, tag="x1f")
                        x1v = x1f[:].rearrange("p (h w) -> p h w", h=c.H)
                        rb = rdn[:, :, None].broadcast_to([128, c.H, c.C])
                        nc.vector.tensor_mul(x1v, pxv, rb)
                        nc.vector.tensor_add(x1f[:], x1f[:],
                                             bgat_sb[:, 0:c.D1])
                        nc.vector.tensor_scalar_max(x1f[:], x1f[:], 0.0)
                        dvb = scl_sb[:, t:t + 1].broadcast_to([128, c.D1])
                        nc.vector.tensor_tensor(yt[:, 0:c.D1], x1f[:], dvb,
                                                Alu.mult)
                    nc.sync.dma_start(y_d[t * 128:(t + 1) * 128, :], yt[:])
                    # chunked AllGather as soon as a chunk's tiles are done
                    if STAGE >= 3 and (t + 1) % c.TPC == 0:
                        k = (t + 1) // c.TPC - 1
                        r0 = k * c.TPC * 128
                        r1 = (k + 1) * c.TPC * 128
                        nc.gpsimd.collective_compute(
                            "AllGather", Alu.bypass,
                            ins=[y_d[r0:r1, :]],
                            outs=[yf_d[r0 * c.NCORES:r1 * c.NCORES, :]],
                            replica_groups=rg)

            if DEBUG:
                nc.sync.dma_start(dbg_h[:], h_d[:])
                nc.sync.dma_start(dbg_y[:], yf_d[:].bitcast(mybir.dt.uint8))

            # ============ Phase 2: GCN scatter + dense + pooling ============
            with tc.tile_pool(name="p2", bufs=2) as p2, \
                 tc.tile_pool(name="p2h", bufs=3) as p2h, \
                 tc.tile_pool(name="p2o", bufs=2) as p2o, \
                 tc.tile_pool(name="gd", bufs=1) as gd, \
                 tc.tile_pool(name="gw", bufs=2) as gw, \
                 tc.tile_pool(name="psW", bufs=2, space="PSUM") as psW, \
                 tc.tile_pool(name="psP", bufs=1, space="PSUM") as psP:
                gacc = pp.tile([128, c.KS * c.G], F32)
                nc.vector.memset(gacc[:], 0.0)

                for hf in range(c.NHALF if STAGE >= 4 else 0):
                    for t in range(hf * c.HT, (hf + 1) * c.HT):
                        cs = t * c.TCT * 8
                        ce = (t + 1) * c.TCT * 8
                        xi = p2.tile([128, c.TCT * 8], I16, tag="xi")
                        nc.sync.dma_start(xi[:], yidx[:, cs:ce])
                        ob2 = p2o.tile([128, c.TCT, 128], F8, tag="ob2")
                        nc.sync.dma_start(ob2[:],
                                          ohb2[:, t * c.TCT:(t + 1) * c.TCT, :])
                        px2 = psA_pool.tile([128, c.DP], F32, tag="px")
                        for g in range(c.NGRP):
                            c0 = g * c.GRP
                            c1 = min(c0 + c.GRP, c.TCT)
                            nch = c1 - c0
                            yg = p2h.tile([128, c.GRP, c.DP], F8, tag="hg")
                            nc.gpsimd.dma_gather(yg[:, 0:nch, :], yf_d[:],
                                                 xi[:, c0 * 8:c1 * 8],
                                                 nch * 128, nch * 128, c.DP)
                            for ch2 in range(c0, c1, 2):
                                first = (ch2 == 0)
                                last = (ch2 == c.TCT - 2)
                                for j0 in range(0, c.DP, 512):
                                    nc.tensor.matmul(
                                        px2[:, j0:j0 + 512],
                                        ob2[:, ch2:ch2 + 2, :],
                                        yg[:, ch2 - c0:ch2 - c0 + 2,
                                           j0:j0 + 512],
                                        start=first, stop=last, perf_mode=DR)
                        agt = p2.tile([128, c.DP], BF16, tag="agt")
                        nc.scalar.copy(agt[:], px2[:])
                        nc.sync.dma_start(aggb_d[t * 128:(t + 1) * 128, :],
                                          agt[:])
                    if STAGE < 5:
                        continue
                    # dense for this half: transpose agg, matmul, relu*dinv
                    hr0 = hf * c.HT * 128
                    hr1 = (hf + 1) * c.HT * 128
                    aggT = gd.tile([128, c.KS, c.HT * 128], BF16, tag="aT")
                    for k in range(c.KS):
                        nc.sync.dma_start(aggT[:, k, :],
                                          aggb_d[hr0:hr1, k * 128:(k + 1) * 128],
                                          transpose=True)
                    for m in range(hf * c.HT, (hf + 1) * c.HT):
                        mo = (m - hf * c.HT) * 128
                        xt2 = gw.tile([128, c.DP], BF16, tag="xt2")
                        for j0 in range(0, c.DP, 512):
                            pw = psW.tile([128, 512], F32, tag="pw")
                            for k in range(c.KS):
                                nc.tensor.matmul(
                                    pw[:],
                                    aggT[:, k, mo:mo + 128],
                                    wgcn_sb[:, k, j0:j0 + 512],
                                    start=(k == 0), stop=(k == c.KS - 1))
                            if not c.B2NZ:
                                nc.scalar.activation(
                                    xt2[:, j0:j0 + 512], pw[:],
                                    Act.Relu, scale=scl_sb[:, m:m + 1])
                            else:
                                xf = gw.tile([128, 512], F32, tag="xf")
                                dvb = scl_sb[:, m:m + 1].broadcast_to(
                                    [128, 512])
                                nc.vector.tensor_tensor(
                                    xf[:], pw[:], dvb, Alu.mult)
                                nc.vector.tensor_add(
                                    xf[:], xf[:], bgcn_sb[:, j0:j0 + 512])
                                nc.vector.tensor_scalar_max(
                                    xt2[:, j0:j0 + 512], xf[:], 0.0)
                        if DEBUG:
                            nc.sync.dma_start(
                                dbg_x2[m * 128:(m + 1) * 128, :], xt2[:])
                        for fb in range(0, c.KS, 8):
                            fe = min(fb + 8, c.KS)
                            pgt = psP.tile([128, 8, c.G], F32, tag="pg")
                            for fs in range(fb, fe):
                                nc.tensor.matmul(
                                    pgt[:, fs - fb, :],
                                    xt2[:, fs * 128:(fs + 1) * 128],
                                    gon_sb[:, m, :],
                                    start=True, stop=True)
                            nc.vector.tensor_add(
                                gacc[:, fb * c.G:fe * c.G],
                                gacc[:, fb * c.G:fe * c.G],
                                pgt[:, 0:fe - fb, :].rearrange(
                                    "p k g -> p (k g)"))
                nc.gpsimd.dma_start(gs_in_d[:], gacc[:])
                if DEBUG:
                    nc.sync.dma_start(dbg_a[:], aggb_d[:])
                    nc.sync.dma_start(dbg_g[:], gs_in_d[:])
            psA.close()

            # ============ AllReduce pooled sums + FC ============
            if STAGE >= 6:
                nc.gpsimd.collective_compute(
                    "AllReduce", Alu.add, ins=[gs_in_d[:]], outs=[gs_out_d[:]],
                    replica_groups=rg)
            with tc.tile_pool(name="fc", bufs=1) as fc, \
                 tc.tile_pool(name="psS", bufs=1, space="PSUM") as psS:
              if STAGE < 6:
                dz = fc.tile([c.G, c.OUT], F32)
                nc.vector.memset(dz[:], 0.0)
                nc.sync.dma_start(out[:], dz[:])
              else:
                gsar = fc.tile([128, c.KS, c.G], F32)
                nc.sync.dma_start(gsar[:],
                                  gs_out_d[:].rearrange("p (k g) -> p k g",
                                                        k=c.KS))
                iv_sb = fc.tile([128, c.G], F32)
                nc.sync.dma_start(iv_sb[:], invcnt[:])
                gm = fc.tile([128, c.KS, c.G], F32)
                nc.vector.tensor_mul(
                    gm[:], gsar[:],
                    iv_sb[:, None, :].broadcast_to([128, c.KS, c.G]))
                wf_sb = fc.tile([128, c.FCK, c.OUT], F32)
                nc.sync.dma_start(
                    wf_sb[:], Wfc[:].rearrange("(k p) o -> p k o", p=128))
                pf = psS.tile([c.G, c.OUT], F32, tag="sm")
                for k in range(c.FCK):
                    lhs = gm[:, k, :] if k < c.KS else gsar[:, k - c.KS, :]
                    nc.tensor.matmul(pf[:], lhs, wf_sb[:, k, :],
                                     start=(k == 0), stop=(k == c.FCK - 1))
                bf_sb = fc.tile([c.G, c.OUT], F32)
                nc.sync.dma_start(bf_sb[:], bfc[:])
                ot = fc.tile([c.G, c.OUT], F32)
                nc.vector.tensor_add(ot[:], pf[:], bf_sb[:])
                nc.vector.tensor_scalar_max(ot[:], ot[:], 0.0)
                nc.sync.dma_start(out[:], ot[:])

    nc.compile()
    return nc


# ================= host-side preprocessing =================

def _wrap_idx(a):
    """[L] int -> [128, L//16] int16 wrapped (i -> [i%16, i//16]) + 8x repl."""
    w = a.reshape(-1, 16).T.astype(np.int16)
    return np.tile(w, (8, 1)).copy()


def preprocess(x, edge_index, batch, num_graphs, W_gat, att_src, att_dst,
               b_gat, W_gcn, b_gcn, W_fc, b_fc, cfg=None, ncores=8):
    N, C = x.shape
    E = edge_index.shape[1]
    H = att_src.shape[0]
    G = int(num_graphs)
    OUT = W_fc.shape[1]

    src = np.concatenate([np.asarray(edge_index[0]), np.arange(N)]).astype(np.int64)
    dst = np.concatenate([np.asarray(edge_index[1]), np.arange(N)]).astype(np.int64)
    deg = np.bincount(dst, minlength=N).astype(np.float32)
    dinv = np.where(deg > 0, 1.0 / np.sqrt(deg), 0.0).astype(np.float32)

    NC_ = ncores
    NPC = _ru(N, NC_) // NC_
    NT = _ru(NPC, 128) // 128

    order = np.argsort(dst, kind='stable')
    s_s, s_d = src[order], dst[order]

    # per (core,tile) edge lists
    tiles = [[None] * NT for _ in range(NC_)]
    for core in range(NC_):
        for t in range(NT):
            lo = np.searchsorted(s_d, core * NPC + t * 128)
            hi = np.searchsorted(s_d, min(core * NPC + (t + 1) * 128,
                                          (core + 1) * NPC))
            tiles[core][t] = (s_s[lo:hi], s_d[lo:hi])

    TCT = max(max(_ru(len(tt[0]), 128) // 128 for tt in row) for row in tiles)
    TCT = max(_ru(TCT, 2), 2)
    if cfg is None:
        cfg = Cfg(N, E, H, C, G, OUT, TCT, NCORES=NC_)
        cfg.B1NZ = bool(np.any(np.asarray(b_gat) != 0))
        cfg.B2NZ = bool(np.any(np.asarray(b_gcn) != 0))
    assert cfg.TCT == TCT

    c = cfg
    # replicated tensors
    xT = np.zeros((C, c.NPAD), BF)
    xT[:, :N] = np.asarray(x).T.astype(BF)
    Wgf = np.asarray(W_gat).astype(np.float32)
    Wg = Wgf.astype(BF)
    Wg3 = Wgf.reshape(C, H, C)
    Mcat = np.zeros((C, 2 * H), BF)
    Mcat[:, 0:H] = np.einsum('khc,hc->kh', Wg3, np.asarray(att_src)).astype(BF)
    Mcat[:, H:2 * H] = np.einsum('khc,hc->kh', Wg3, np.asarray(att_dst)).astype(BF)
    bgat = np.zeros((128, c.DP), np.float32)
    bgat[:, :c.D1] = np.asarray(b_gat)[None, :]
    bgcn = np.zeros((128, c.DP), np.float32)
    bgcn[:, :c.D1] = np.asarray(b_gcn)[None, :]
    Wgcn = np.zeros((c.DP, c.DP), BF)
    Wgcn[:c.D1, :c.D1] = np.asarray(W_gcn).astype(BF)
    Wfc = np.zeros((2 * c.DP, OUT), np.float32)
    Wfc[0:c.D1] = np.asarray(W_fc)[0:c.D1]
    Wfc[c.DP:c.DP + c.D1] = np.asarray(W_fc)[c.D1:2 * c.D1]
    bfc = np.tile(np.asarray(b_fc).astype(np.float32)[None, :], (G, 1))
    cnt = np.bincount(np.asarray(batch), minlength=G).astype(np.float32)
    invcnt = np.tile((1.0 / np.maximum(cnt, 1.0))[None, :], (128, 1))

    batch_np = np.asarray(batch)
    shared = dict(xT=xT, Wg=Wg, Mcat=Mcat, Wgcn=Wgcn, Wfc=Wfc, bfc=bfc,
                  invcnt=invcnt, bgat=bgat, bgcn=bgcn)

    # y row index in the chunk-wise AllGathered layout, per source node id
    def yrow_of(j, core_of):
        local = j - core_of * NPC
        t = local // 128
        r = local % 128
        k = t // c.TPC
        return (k * c.NCORES * c.TPC * 128 + core_of * c.TPC * 128
                + (t - k * c.TPC) * 128 + r)

    in_maps = []
    for core in range(NC_):
        L = c.TC * 128
        sp = np.zeros(L, np.int64)
        dl = np.zeros(L, np.int64)
        valid = np.zeros(L, bool)
        for t in range(NT):
            ts, td = tiles[core][t]
            o = t * c.TCT * 128
            k = len(ts)
            sp[o:o + k] = ts
            dl[o:o + k] = td - (core * NPC + t * 128)
            valid[o:o + k] = True
        cs = sp // NPC
        yr = np.array([yrow_of(j, cj) for j, cj in zip(sp, cs)], np.int64)
        oh = np.zeros((c.TC, 128, 128), np.float32)
        ee = np.arange(L)
        oh[ee // 128, ee % 128, dl] = valid.astype(np.float32)
        # this core's dst-node ids per (tile, slot), clamped to valid rows
        dnids = np.zeros((NT, 128), np.int64)
        for t in range(NT):
            gids = core * NPC + t * 128 + np.arange(128)
            dnids[t] = np.minimum(gids, N - 1)
        sclm = np.zeros((128, NT), np.float32)
        for t in range(NT):
            gids = core * NPC + t * 128 + np.arange(128)
            ok = gids < min((core + 1) * NPC, N)
            sclm[ok, t] = dinv[gids[ok]]
        gonm = np.zeros((128, NT, G), BF)
        for t in range(NT):
            gids = core * NPC + t * 128 + np.arange(128)
            ok = gids < min((core + 1) * NPC, N)
            gonm[ok, t, batch_np[gids[ok]]] = 1.0
        m = dict(shared)
        m.update(
            sidx=_wrap_idx(sp), yidx=_wrap_idx(yr),
            dnid=_wrap_idx(dnids.reshape(-1)),
            ohb1=oh.transpose(1, 0, 2).astype(BF),
            ohb2=oh.transpose(1, 0, 2).astype(NPF8),
            ohT=oh.transpose(2, 0, 1).astype(BF),
            scl=sclm, gon=gonm)
        in_maps.append(m)
    return cfg, in_maps


_CACHE = {}


def run(inputs, trace=False):
    key = tuple(sorted((k, tuple(np.shape(v))) for k, v in inputs.items()))
    cfg, in_maps = preprocess(**inputs,
                              cfg=_CACHE[key][0] if key in _CACHE else None)
    if key not in _CACHE:
        _CACHE[key] = (cfg, build(cfg))
    cfg, nc = _CACHE[key]
    res = run_bass_kernel_spmd(nc, in_maps, core_ids=list(range(cfg.NCORES)),
                               trace=trace)
    return res.results[0]["out"].astype(np.float32), res


def kernel(**inputs):
    out, _ = run(inputs)
    return out


# revision 52
# speedup vs baseline: 1.3368x; 1.0062x over previous
"""GAT+GCN+pool GNN on 8 Trainium2 NeuronCores (Bass/Tile), fp8 edition.

Sharding: nodes/edges partitioned across 8 cores by destination-node range;
segment softmax and scatter-adds are core-local.  Per-edge row gathers use
dma_gather on fp8 rows (h stored as [2496 h | 32 a_src | 32 a_dst] fp8e4),
scatter-adds are DoubleRow fp8 one-hot matmuls (256 edges per pass).

GCN is computed as (A_hat x1) W (associativity) so the only big exchange is
an AllGather of the dinv-prescaled GAT output y = dinv*x1 in fp8 (26MB),
issued in chunks overlapped with phase-1 compute.  The same one-hot tensor
drives both scatter phases.  Dense GCN (bf16) runs per half-graph interleaved
with phase-2 scatter; graph pooling accumulates in PSUM across tiles.

Pipeline (per core, one NEFF):
  A)  h = x @ W_gat (bf16, replicated), a_src/a_dst folded matmul -> fp8 h_d
  1)  per dst-tile: gather fp8 rows per edge -> logits -> exp ->
      exp*h via DVE+GpSimd split -> DoubleRow one-hot scatter -> y (fp8)
  AG) chunked AllGather of y
  2)  per half: gather y rows, DoubleRow one-hot scatter -> agg; DMA-transpose;
      dense agg @ W_gcn (bf16) with fused relu*dinv; pooling matmul in PSUM
  AR) AllReduce pooled sums, gmean, FC, relu -> out [G, OUT]
"""

import sys
import os
import contextlib

if '/opt/trn_rl_repo' not in sys.path:
    sys.path.insert(0, '/opt/trn_rl_repo')

import numpy as np
import ml_dtypes

import concourse.bacc as bacc
import concourse.mybir as mybir
import concourse.tile as tile
from concourse.bass_utils import run_bass_kernel_spmd

F32 = mybir.dt.float32
BF16 = mybir.dt.bfloat16
F8 = mybir.dt.float8e4
I16 = mybir.dt.int16
BF = ml_dtypes.bfloat16
NPF8 = ml_dtypes.float8_e4m3
Alu = mybir.AluOpType
Act = mybir.ActivationFunctionType
DR = mybir.MatmulPerfMode.DoubleRow


def _ru(x, m):
    return (x + m - 1) // m * m


class Cfg:
    def __init__(self, N, E, H, C, G, OUT, TCT, NCORES=8, GRP=6, HD=22, AGC=1):
        self.N, self.E, self.H, self.C, self.G, self.OUT = N, E, H, C, G, OUT
        self.NCORES = NCORES
        self.D1 = H * C                              # 2496
        self.DP = _ru(self.D1 + 2 * H, 128)          # 2560 fp8 row bytes
        assert self.DP % 256 == 0
        self.NPC = _ru(N, NCORES) // NCORES          # nodes per core
        self.NT = _ru(self.NPC, 128) // 128          # dst tiles per core
        self.XWROWS = self.NT * 128
        self.XWFULL = NCORES * self.XWROWS
        self.ROWS_A = _ru(N, 128) // 128             # stage-A node tiles
        self.NPAD = self.ROWS_A * 128
        self.KS = self.DP // 128                     # dense k slabs
        self.FCK = 2 * self.KS
        assert TCT % 2 == 0
        self.TCT = TCT                               # chunks per dst tile
        self.TC = self.NT * TCT
        self.GRP = GRP                               # chunks per gather group
        assert GRP % 2 == 0
        self.NGRP = (TCT + GRP - 1) // GRP
        self.HD = HD                                 # heads multiplied on DVE
        self.AGC = AGC                               # allgather chunks
        assert self.NT % AGC == 0
        self.TPC = self.NT // AGC                    # tiles per AG chunk
        self.NHALF = 2                               # dense half-phases
        assert self.NT % self.NHALF == 0
        self.HT = self.NT // self.NHALF              # tiles per half
        self.B1NZ = False                            # b_gat nonzero
        self.B2NZ = False                            # b_gcn nonzero


def build(cfg):
    STAGE = int(os.environ.get("GNN_STAGE", "9"))
    DEBUG = int(os.environ.get("GNN_DEBUG", "0"))
    MV = int(os.environ.get("GNN_MV", "0"))
    CD = int(os.environ.get("GNN_CD", "4"))
    hd_env = os.environ.get("GNN_HD")
    if hd_env is not None:
        cfg.HD = int(hd_env)
    c = cfg
    nc = bacc.Bacc(None, target_bir_lowering=False)

    # ---- external inputs (replicated unless noted per-core) ----
    xT = nc.dram_tensor("xT", [c.C, c.NPAD], BF16, kind="ExternalInput")
    Wg = nc.dram_tensor("Wg", [c.C, c.D1], BF16, kind="ExternalInput")
    Mcat = nc.dram_tensor("Mcat", [c.C, 2 * c.H], BF16, kind="ExternalInput")
    Wgcn = nc.dram_tensor("Wgcn", [c.DP, c.DP], BF16, kind="ExternalInput")
    Wfc = nc.dram_tensor("Wfc", [2 * c.DP, c.OUT], F32, kind="ExternalInput")
    bfc = nc.dram_tensor("bfc", [c.G, c.OUT], F32, kind="ExternalInput")
    bgat = nc.dram_tensor("bgat", [128, c.DP], F32, kind="ExternalInput")
    bgcn = nc.dram_tensor("bgcn", [128, c.DP], F32, kind="ExternalInput")
    invcnt = nc.dram_tensor("invcnt", [128, c.G], F32, kind="ExternalInput")
    # per-core:
    sidx = nc.dram_tensor("sidx", [128, c.TC * 8], I16, kind="ExternalInput")
    yidx = nc.dram_tensor("yidx", [128, c.TC * 8], I16, kind="ExternalInput")
    dnid = nc.dram_tensor("dnid", [128, c.NT * 8], I16, kind="ExternalInput")
    ohb1 = nc.dram_tensor("ohb1", [128, c.TC, 128], BF16, kind="ExternalInput")
    ohb2 = nc.dram_tensor("ohb2", [128, c.TC, 128], F8, kind="ExternalInput")
    ohT = nc.dram_tensor("ohT", [128, c.TC, 128], BF16, kind="ExternalInput")
    scl = nc.dram_tensor("scl", [128, c.NT], F32, kind="ExternalInput")
    gon = nc.dram_tensor("gon", [128, c.NT, c.G], BF16, kind="ExternalInput")
    out = nc.dram_tensor("out", [c.G, c.OUT], F32, kind="ExternalOutput")
    if DEBUG:
        dbg_h = nc.dram_tensor("dbg_h", [c.NPAD, c.DP], BF16,
                               kind="ExternalOutput")
        dbg_y = nc.dram_tensor("dbg_y", [c.XWFULL, c.DP], mybir.dt.uint8,
                               kind="ExternalOutput")
        dbg_a = nc.dram_tensor("dbg_a", [c.XWROWS, c.DP], BF16,
                               kind="ExternalOutput")
        dbg_g = nc.dram_tensor("dbg_g", [128, c.KS * c.G], F32,
                               kind="ExternalOutput")
        dbg_x2 = nc.dram_tensor("dbg_x2", [c.XWROWS, c.DP], BF16,
                                kind="ExternalOutput")

    rg = [list(range(c.NCORES))]

    with tile.TileContext(nc) as tc:
        with (
            tc.tile_pool(name="dram", bufs=1, space="DRAM") as dram,
            tc.tile_pool(name="persist", bufs=1) as pp,
        ):
            h_d = dram.tile([c.NPAD, c.DP], BF16)
            y_d = dram.tile([c.XWROWS, c.DP], F8)
            yf_d = dram.tile([c.XWFULL, c.DP], F8, addr_space="Shared")
            aggb_d = dram.tile([c.XWROWS, c.DP], BF16)
            gs_in_d = dram.tile([128, c.KS * c.G], F32)
            gs_out_d = dram.tile([128, c.KS * c.G], F32, addr_space="Shared")

            # persistent smalls + resident GCN weights
            scl_sb = pp.tile([128, c.NT], F32)
            nc.sync.dma_start(scl_sb[:], scl[:])
            c02 = pp.tile([128, 1], BF16)
            nc.vector.memset(c02[:], 0.2)
            gon_sb = pp.tile([128, c.NT, c.G], BF16)
            nc.sync.dma_start(gon_sb[:], gon[:])
            wgcn_sb = pp.tile([128, c.KS, c.DP], BF16)
            for k in range(c.KS):
                nc.sync.dma_start(wgcn_sb[:, k, :],
                                  Wgcn[k * 128:(k + 1) * 128, :])
            if c.B1NZ:
                bgat_sb = pp.tile([128, c.DP], F32)
                nc.sync.dma_start(bgat_sb[:], bgat[:])
            if c.B2NZ:
                bgcn_sb = pp.tile([128, c.DP], F32)
                nc.sync.dma_start(bgcn_sb[:], bgcn[:])

            # ============ Stage A: h = x@Wg -> fp8 h_d with a-tail ============
            with tc.tile_pool(name="stageA", bufs=3) as sa, \
                 tc.tile_pool(name="stageAc", bufs=1) as sac, \
                 tc.tile_pool(name="psH", bufs=5, space="PSUM") as psH, \
                 tc.tile_pool(name="psHa", bufs=2, space="PSUM") as psHa:
                xT_sb = sac.tile([c.C, c.NPAD], BF16)
                nc.sync.dma_start(xT_sb[:], xT[:])
                Wg_sb = sac.tile([c.C, c.D1], BF16)
                nc.sync.dma_start(Wg_sb[:], Wg[:])
                Mc_sb = sac.tile([c.C, 2 * c.H], BF16)
                nc.sync.dma_start(Mc_sb[:], Mcat[:])
                for r in range(c.ROWS_A if STAGE >= 1 else 0):
                    lhs = xT_sb[:, r * 128:(r + 1) * 128]
                    hb = sa.tile([128, c.DP], BF16, tag="hb")
                    for i, j0 in enumerate(range(0, c.D1, 512)):
                        j1 = min(j0 + 512, c.D1)
                        ph = psH.tile([128, 512], F32, tag="ph")
                        nc.tensor.matmul(ph[:, 0:j1 - j0], lhs, Wg_sb[:, j0:j1],
                                         start=True, stop=True)
                        if i % 2 == 0:
                            nc.scalar.copy(hb[:, j0:j1], ph[:, 0:j1 - j0])
                        else:
                            nc.vector.tensor_copy(hb[:, j0:j1], ph[:, 0:j1 - j0])
                    pa = psHa.tile([128, 2 * c.H], F32, tag="pa")
                    nc.tensor.matmul(pa[:], lhs, Mc_sb[:], start=True, stop=True)
                    nc.vector.tensor_copy(hb[:, c.D1:c.D1 + 2 * c.H], pa[:])
                    nc.sync.dma_start(h_d[r * 128:(r + 1) * 128, :], hb[:])

            psA = contextlib.ExitStack()
            psA_pool = psA.enter_context(
                tc.tile_pool(name="psA", bufs=1, space="PSUM"))

            # ============ Phase 1: GAT edge softmax + scatter -> y ============
            with tc.tile_pool(name="p1", bufs=2) as p1, \
                 tc.tile_pool(name="p1h", bufs=2) as p1h, \
                 tc.tile_pool(name="p1o", bufs=2) as p1o, \
                 tc.tile_pool(name="psD", bufs=1, space="PSUM") as psD, \
                 tc.tile_pool(name="psE", bufs=2, space="PSUM") as psE:
                for t in range(c.NT if STAGE >= 2 else 0):
                    cs = t * c.TCT * 8
                    ce = (t + 1) * c.TCT * 8
                    si = p1.tile([128, c.TCT * 8], I16, tag="si")
                    nc.sync.dma_start(si[:], sidx[:, cs:ce])
                    ob = p1o.tile([128, c.TCT, 128], F8, tag="ob")
                    nc.sync.dma_start(ob[:], ohb2[:, t * c.TCT:(t + 1) * c.TCT, :])
                    obw = p1o.tile([128, c.TCT, 128], BF16, tag="obw")
                    nc.sync.dma_start(obw[:], ohb1[:, t * c.TCT:(t + 1) * c.TCT, :])
                    oT = p1o.tile([128, c.TCT, 128], BF16, tag="oT")
                    nc.sync.dma_start(oT[:], ohT[:, t * c.TCT:(t + 1) * c.TCT, :])
                    dn = p1.tile([128, 8], I16, tag="dn")
                    nc.sync.dma_start(dn[:], dnid[:, t * 8:(t + 1) * 8])
                    adt = p1.tile([128, 1, 128], BF16, tag="adt")
                    nc.gpsimd.dma_gather(adt[:], h_d[:, c.DP - 128:c.DP],
                                         dn[:], 128, 128, 128, elem_step=c.DP)

                    px = psA_pool.tile([128, c.DP], F32, tag="px")
                    pd = psD.tile([128, c.H], F32, tag="pd")
                    exf = p1.tile([128, c.TCT, c.H], BF16, tag="exf")
                    exf2 = p1.tile([128, c.TCT, c.H], BF16, tag="exf2")
                    ex8 = p1.tile([128, c.TCT, c.H], F8, tag="ex8")
                    for g in range(c.NGRP):
                        c0 = g * c.GRP
                        c1 = min(c0 + c.GRP, c.TCT)
                        nch = c1 - c0
                        hg = p1h.tile([128, c.GRP, c.DP], BF16, tag="hg")
                        nc.gpsimd.dma_gather(hg[:, 0:nch, :], h_d[:],
                                             si[:, c0 * 8:c1 * 8],
                                             nch * 128, nch * 128, c.DP)
                        peg = psE.tile([128, c.GRP, c.H], F32, tag="peg")
                        for ch in range(c0, c1):
                            nc.tensor.matmul(
                                peg[:, ch - c0, :], oT[:, ch, :],
                                adt[:, 0, 128 - c.H:128],
                                start=True, stop=True)
                        ev = exf[:, c0:c1, :]
                        nc.vector.tensor_add(ev, peg[:, 0:nch, :],
                                             hg[:, 0:nch, c.D1:c.D1 + c.H])
                        ev2 = exf2[:, c0:c1, :]
                        nc.vector.tensor_tensor(
                            ev2, ev,
                            c02[:, :, None].broadcast_to([128, nch, c.H]),
                            Alu.mult)
                        nc.vector.tensor_tensor(ev, ev, ev2, Alu.max)
                        nc.scalar.activation(ev, ev, Act.Exp)
                        nc.scalar.copy(ex8[:, c0:c1, :], ev)
                        mv = hg[:, 0:nch, 0:c.D1].rearrange(
                            "p t (h w) -> p t h w", h=c.H)
                        ebl = exf[:, c0:c1, 0:c.HD, None].broadcast_to(
                            [128, nch, c.HD, c.C])
                        ebp = exf[:, c0:c1, c.HD:c.H, None].broadcast_to(
                            [128, nch, c.H - c.HD, c.C])
                        nc.vector.tensor_mul(mv[:, :, 0:c.HD, :],
                                             mv[:, :, 0:c.HD, :], ebl)
                        if c.HD < c.H:
                            nc.gpsimd.tensor_mul(mv[:, :, c.HD:c.H, :],
                                                 mv[:, :, c.HD:c.H, :], ebp)
                        for ch2 in range(c0, c1, 2):
                            first = (ch2 == 0)
                            last = (ch2 == c.TCT - 2)
                            nc.tensor.matmul(pd[:], ob[:, ch2:ch2 + 2, :],
                                             ex8[:, ch2:ch2 + 2, :],
                                             start=first, stop=last,
                                             perf_mode=DR)
                        for ch in range(c0, c1):
                            for j0 in range(0, c.DP, 512):
                                nc.tensor.matmul(
                                    px[:, j0:j0 + 512], obw[:, ch, :],
                                    hg[:, ch - c0, j0:j0 + 512],
                                    start=(ch == 0), stop=(ch == c.TCT - 1))
                    rdn = p1.tile([128, c.H], F32, tag="rdn")
                    nc.vector.reciprocal(rdn[:], pd[:])
                    sc = p1.tile([128, c.H], F32, tag="sc")
                    nc.vector.tensor_mul(
                        sc[:], rdn[:],
                        scl_sb[:, t:t + 1].broadcast_to([128, c.H]))
                    yt = p1.tile([128, c.DP], F8, tag="yt")
                    nc.vector.memset(yt[:, c.D1:], 0.0)
                    pxv = px[:, 0:c.D1].rearrange("p (h w) -> p h w", h=c.H)
                    ytv = yt[:, 0:c.D1].rearrange("p (h w) -> p h w", h=c.H)
                    scb = sc[:, :, None].broadcast_to([128, c.H, c.C])
                    if not c.B1NZ:
                        nc.vector.scalar_tensor_tensor(ytv, pxv, 0.0, scb,
                                                       Alu.max, Alu.mult)
                    else:
                        x1f = p1.tile([128, c.D1], F32, tag="x1f")
                        x1v = x1f[:].rearrange("p (h w) -> p h w", h=c.H)
                        rb = rdn[:, :, None].broadcast_to([128, c.H, c.C])
                        nc.vector.tensor_mul(x1v, pxv, rb)
                        nc.vector.tensor_add(x1f[:], x1f[:],
                                             bgat_sb[:, 0:c.D1])
                        nc.vector.tensor_scalar_max(x1f[:], x1f[:], 0.0)
                        dvb = scl_sb[:, t:t + 1].broadcast_to([128, c.D1])
                        nc.vector.tensor_tensor(yt[:, 0:c.D1], x1f[:], dvb,
                                                Alu.mult)
                    nc.sync.dma_start(y_d[t * 128:(t + 1) * 128, :], yt[:])
                    # chunked AllGather as soon as a chunk's tiles are done
                    if STAGE >= 3 and (t + 1) % c.TPC == 0:
                        k = (t + 1) // c.TPC - 1
                        r0 = k * c.TPC * 128
                        r1 = (k + 1) * c.TPC * 128
                        nc.gpsimd.collective_compute(
                            "AllGather", Alu.bypass,
                            ins=[y_d[r0:r1, :]],
                            outs=[yf_d[r0 * c.NCORES:r1 * c.NCORES, :]],
                            replica_groups=rg)

            if DEBUG:
                nc.sync.dma_start(dbg_h[:], h_d[:])
                nc.sync.dma_start(dbg_y[:], yf_d[:].bitcast(mybir.dt.uint8))

            # ============ Phase 2: GCN scatter + dense + pooling ============
            with tc.tile_pool(name="p2", bufs=2) as p2, \
                 tc.tile_pool(name="p2h", bufs=3) as p2h, \
                 tc.tile_pool(name="p2o", bufs=2) as p2o, \
                 tc.tile_pool(name="gd", bufs=1) as gd, \
                 tc.tile_pool(name="gw", bufs=2) as gw, \
                 tc.tile_pool(name="psW", bufs=2, space="PSUM") as psW, \
                 tc.tile_pool(name="psP", bufs=1, space="PSUM") as psP:
                gacc = pp.tile([128, c.KS * c.G], F32)
                nc.vector.memset(gacc[:], 0.0)

                for hf in range(c.NHALF if STAGE >= 4 else 0):
                    for t in range(hf * c.HT, (hf + 1) * c.HT):
                        cs = t * c.TCT * 8
                        ce = (t + 1) * c.TCT * 8
                        xi = p2.tile([128, c.TCT * 8], I16, tag="xi")
                        nc.sync.dma_start(xi[:], yidx[:, cs:ce])
                        ob2 = p2o.tile([128, c.TCT, 128], F8, tag="ob2")
                        nc.sync.dma_start(ob2[:],
                                          ohb2[:, t * c.TCT:(t + 1) * c.TCT, :])
                        px2 = psA_pool.tile([128, c.DP], F32, tag="px")
                        for g in range(c.NGRP):
                            c0 = g * c.GRP
                            c1 = min(c0 + c.GRP, c.TCT)
                            nch = c1 - c0
                            yg = p2h.tile([128, c.GRP, c.DP], F8, tag="hg")
                            nc.gpsimd.dma_gather(yg[:, 0:nch, :], yf_d[:],
                                                 xi[:, c0 * 8:c1 * 8],
                                                 nch * 128, nch * 128, c.DP)
                            for ch2 in range(c0, c1, 2):
                                first = (ch2 == 0)
                                last = (ch2 == c.TCT - 2)
                                for j0 in range(0, c.DP, 512):
                                    nc.tensor.matmul(
                                        px2[:, j0:j0 + 512],
                                        ob2[:, ch2:ch2 + 2, :],
                                        yg[:, ch2 - c0:ch2 - c0 + 2,
                                           j0:j0 + 512],
                                        start=first, stop=last, perf_mode=DR)
                        agt = p2.tile([128, c.DP], BF16, tag="agt")
                        nc.scalar.copy(agt[:], px2[:])
                        nc.sync.dma_start(aggb_d[t * 128:(t + 1) * 128, :],
                                          agt[:])
                    if STAGE < 5:
                        continue
                    # dense for this half: transpose agg, matmul, relu*dinv
                    hr0 = hf * c.HT * 128
                    hr1 = (hf + 1) * c.HT * 128
                    aggT = gd.tile([128, c.KS, c.HT * 128], BF16, tag="aT")
                    for k in range(c.KS):
                        nc.sync.dma_start(aggT[:, k, :],
                                          aggb_d[hr0:hr1, k * 128:(k + 1) * 128],
                                          transpose=True)
                    for m in range(hf * c.HT, (hf + 1) * c.HT):
                        mo = (m - hf * c.HT) * 128
                        xt2 = gw.tile([128, c.DP], BF16, tag="xt2")
                        for j0 in range(0, c.DP, 512):
                            pw = psW.tile([128, 512], F32, tag="pw")
                            for k in range(c.KS):
                                nc.tensor.matmul(
                                    pw[:],
                                    aggT[:, k, mo:mo + 128],
                                    wgcn_sb[:, k, j0:j0 + 512],
                                    start=(k == 0), stop=(k == c.KS - 1))
                            if not c.B2NZ:
                                nc.scalar.activation(
                                    xt2[:, j0:j0 + 512], pw[:],
                                    Act.Relu, scale=scl_sb[:, m:m + 1])
                            else:
                                xf = gw.tile([128, 512], F32, tag="xf")
                                dvb = scl_sb[:, m:m + 1].broadcast_to(
                                    [128, 512])
                                nc.vector.tensor_tensor(
                                    xf[:], pw[:], dvb, Alu.mult)
                                nc.vector.tensor_add(
                                    xf[:], xf[:], bgcn_sb[:, j0:j0 + 512])
                                nc.vector.tensor_scalar_max(
                                    xt2[:, j0:j0 + 512], xf[:], 0.0)
                        if DEBUG:
                            nc.sync.dma_start(
                                dbg_x2[m * 128:(m + 1) * 128, :], xt2[:])
                        for fb in range(0, c.KS, 8):
                            fe = min(fb + 8, c.KS)
                            pgt = psP.tile([128, 8, c.G], F32, tag="pg")
                            for fs in range(fb, fe):
                                nc.tensor.matmul(
                                    pgt[:, fs - fb, :],
                                    xt2[:, fs * 128:(fs + 1) * 128],
                                    gon_sb[:, m, :],
                                    start=True, stop=True)
                            nc.vector.tensor_add(
                                gacc[:, fb * c.G:fe * c.G],
                                gacc[:, fb * c.G:fe * c.G],
                                pgt[:, 0:fe - fb, :].rearrange(
                                    "p k g -> p (k g)"))
                nc.gpsimd.dma_start(gs_in_d[:], gacc[:])
                if DEBUG:
                    nc.sync.dma_start(dbg_a[:], aggb_d[:])
                    nc.sync.dma_start(dbg_g[:], gs_in_d[:])
            psA.close()

            # ============ AllReduce pooled sums + FC ============
            if STAGE >= 6:
                nc.gpsimd.collective_compute(
                    "AllReduce", Alu.add, ins=[gs_in_d[:]], outs=[gs_out_d[:]],
                    replica_groups=rg)
            with tc.tile_pool(name="fc", bufs=1) as fc, \
                 tc.tile_pool(name="psS", bufs=1, space="PSUM") as psS:
              if STAGE < 6:
                dz = fc.tile([c.G, c.OUT], F32)
                nc.vector.memset(dz[:], 0.0)
                nc.sync.dma_start(out[:], dz[:])
              else:
                gsar = fc.tile([128, c.KS, c.G], F32)
                nc.sync.dma_start(gsar[:],
                                  gs_out_d[:].rearrange("p (k g) -> p k g",
                                                        k=c.KS))
                iv_sb = fc.tile([128, c.G], F32)
                nc.sync.dma_start(iv_sb[:], invcnt[:])
                gm = fc.tile([128, c.KS, c.G], F32)
                nc.vector.tensor_mul(
                    gm[:], gsar[:],
                    iv_sb[:, None, :].broadcast_to([128, c.KS, c.G]))
                wf_sb = fc.tile([128, c.FCK, c.OUT], F32)
                nc.sync.dma_start(
                    wf_sb[:], Wfc[:].rearrange("(k p) o -> p k o", p=128))
                pf = psS.tile([c.G, c.OUT], F32, tag="sm")
                for k in range(c.FCK):
                    lhs = gm[:, k, :] if k < c.KS else gsar[:, k - c.KS, :]
                    nc.tensor.matmul(pf[:], lhs, wf_sb[:, k, :],
                                     start=(k == 0), stop=(k == c.FCK - 1))
                bf_sb = fc.tile([c.G, c.OUT], F32)
                nc.sync.dma_start(bf_sb[:], bfc[:])
                ot = fc.tile([c.G, c.OUT], F32)
                nc.vector.tensor_add(ot[:], pf[:], bf_sb[:])
                nc.vector.tensor_scalar_max(ot[:], ot[:], 0.0)
                nc.sync.dma_start(out[:], ot[:])

    nc.compile()
    return nc


# ================= host-side preprocessing =================

def _wrap_idx(a):
    """[L] int -> [128, L//16] int16 wrapped (i -> [i%16, i//16]) + 8x repl."""
    w = a.reshape(-1, 16).T.astype(np.int16)
    return np.tile(w, (8, 1)).copy()


def preprocess(x, edge_index, batch, num_graphs, W_gat, att_src, att_dst,
               b_gat, W_gcn, b_gcn, W_fc, b_fc, cfg=None, ncores=8):
    N, C = x.shape
    E = edge_index.shape[1]
    H = att_src.shape[0]
    G = int(num_graphs)
    OUT = W_fc.shape[1]

    src = np.concatenate([np.asarray(edge_index[0]), np.arange(N)]).astype(np.int64)
    dst = np.concatenate([np.asarray(edge_index[1]), np.arange(N)]).astype(np.int64)
    deg = np.bincount(dst, minlength=N).astype(np.float32)
    dinv = np.where(deg > 0, 1.0 / np.sqrt(deg), 0.0).astype(np.float32)

    NC_ = ncores
    NPC = _ru(N, NC_) // NC_
    NT = _ru(NPC, 128) // 128

    order = np.argsort(dst, kind='stable')
    s_s, s_d = src[order], dst[order]

    # per (core,tile) edge lists
    tiles = [[None] * NT for _ in range(NC_)]
    for core in range(NC_):
        for t in range(NT):
            lo = np.searchsorted(s_d, core * NPC + t * 128)
            hi = np.searchsorted(s_d, min(core * NPC + (t + 1) * 128,
                                          (core + 1) * NPC))
            tiles[core][t] = (s_s[lo:hi], s_d[lo:hi])

    TCT = max(max(_ru(len(tt[0]), 128) // 128 for tt in row) for row in tiles)
    TCT = max(_ru(TCT, 2), 2)
    if cfg is None:
        cfg = Cfg(N, E, H, C, G, OUT, TCT, NCORES=NC_)
        cfg.B1NZ = bool(np.any(np.asarray(b_gat) != 0))
        cfg.B2NZ = bool(np.any(np.asarray(b_gcn) != 0))
    assert cfg.TCT == TCT

    c = cfg
    # replicated tensors
    xT = np.zeros((C, c.NPAD), BF)
    xT[:, :N] = np.asarray(x).T.astype(BF)
    Wgf = np.asarray(W_gat).astype(np.float32)
    Wg = Wgf.astype(BF)
    Wg3 = Wgf.reshape(C, H, C)
    Mcat = np.zeros((C, 2 * H), BF)
    Mcat[:, 0:H] = np.einsum('khc,hc->kh', Wg3, np.asarray(att_src)).astype(BF)
    Mcat[:, H:2 * H] = np.einsum('khc,hc->kh', Wg3, np.asarray(att_dst)).astype(BF)
    bgat = np.zeros((128, c.DP), np.float32)
    bgat[:, :c.D1] = np.asarray(b_gat)[None, :]
    bgcn = np.zeros((128, c.DP), np.float32)
    bgcn[:, :c.D1] = np.asarray(b_gcn)[None, :]
    Wgcn = np.zeros((c.DP, c.DP), BF)
    Wgcn[:c.D1, :c.D1] = np.asarray(W_gcn).astype(BF)
    Wfc = np.zeros((2 * c.DP, OUT), np.float32)
    Wfc[0:c.D1] = np.asarray(W_fc)[0:c.D1]
    Wfc[c.DP:c.DP + c.D1] = np.asarray(W_fc)[c.D1:2 * c.D1]
    bfc = np.tile(np.asarray(b_fc).astype(np.float32)[None, :], (G, 1))
    cnt = np.bincount(np.asarray(batch), minlength=G).astype(np.float32)
    invcnt = np.tile((1.0 / np.maximum(cnt, 1.0))[None, :], (128, 1))

    batch_np = np.asarray(batch)
    shared = dict(xT=xT, Wg=Wg, Mcat=Mcat, Wgcn=Wgcn, Wfc=Wfc, bfc=bfc,
                  invcnt=invcnt, bgat=bgat, bgcn=bgcn)

    # y row index in the chunk-wise AllGathered layout, per source node id
    def yrow_of(j, core_of):
        local = j - core_of * NPC
        t = local // 128
        r = local % 128
        k = t // c.TPC
        return (k * c.NCORES * c.TPC * 128 + core_of * c.TPC * 128
                + (t - k * c.TPC) * 128 + r)

    in_maps = []
    for core in range(NC_):
        L = c.TC * 128
        sp = np.zeros(L, np.int64)
        dl = np.zeros(L, np.int64)
        valid = np.zeros(L, bool)
        for t in range(NT):
            ts, td = tiles[core][t]
            o = t * c.TCT * 128
            k = len(ts)
            sp[o:o + k] = ts
            dl[o:o + k] = td - (core * NPC + t * 128)
            valid[o:o + k] = True
        cs = sp // NPC
        yr = np.array([yrow_of(j, cj) for j, cj in zip(sp, cs)], np.int64)
        oh = np.zeros((c.TC, 128, 128), np.float32)
        ee = np.arange(L)
        oh[ee // 128, ee % 128, dl] = valid.astype(np.float32)
        # this core's dst-node ids per (tile, slot), clamped to valid rows
        dnids = np.zeros((NT, 128), np.int64)
        for t in range(NT):
            gids = core * NPC + t * 128 + np.arange(128)
            dnids[t] = np.minimum(gids, N - 1)
        sclm = np.zeros((128, NT), np.float32)
        for t in range(NT):
            gids = core * NPC + t * 128 + np.arange(128)
            ok = gids < min((core + 1) * NPC, N)
            sclm[ok, t] = dinv[gids[ok]]
        gonm = np.zeros((128, NT, G), BF)
        for t in range(NT):
            gids = core * NPC + t * 128 + np.arange(128)
            ok = gids < min((core + 1) * NPC, N)
            gonm[ok, t, batch_np[gids[ok]]] = 1.0
        m = dict(shared)
        m.update(
            sidx=_wrap_idx(sp), yidx=_wrap_idx(yr),
            dnid=_wrap_idx(dnids.reshape(-1)),
            ohb1=oh.transpose(1, 0, 2).astype(BF),
            ohb2=oh.transpose(1, 0, 2).astype(NPF8),
            ohT=oh.transpose(2, 0, 1).astype(BF),
            scl=sclm, gon=gonm)
        in_maps.append(m)
    return cfg, in_maps


_CACHE = {}


def run(inputs, trace=False):
    key = tuple(sorted((k, tuple(np.shape(v))) for k, v in inputs.items()))
    cfg, in_maps = preprocess(**inputs,
                              cfg=_CACHE[key][0] if key in _CACHE else None)
    if key not in _CACHE:
        _CACHE[key] = (cfg, build(cfg))
    cfg, nc = _CACHE[key]
    res = run_bass_kernel_spmd(nc, in_maps, core_ids=list(range(cfg.NCORES)),
                               trace=trace)
    return res.results[0]["out"].astype(np.float32), res


def kernel(**inputs):
    out, _ = run(inputs)
    return out
